# revision 9
# baseline (speedup 1.0000x reference)
"""Trainium2 Bass kernel for nn_EstCoordNet (PointNet-style loss_fn).

Sharding: data-parallel over B=8 across 8 NeuronCores (one batch item per
core).  BN batch stats are combined with one small AllGather per BN layer;
the final scalar losses with one AllReduce.  Convs run as plain fp32
matmuls; the three 2048x2048 distance matrices run as error-compensated
float32r matmuls (hi/lo split -> ~fp32 accuracy at full PE rate).

Key structural tricks:
  * activations kept channel-major (C, 2048) so BN scale/bias are
    per-partition vectors and one fused ACT Relu applies BN+relu.
  * global max-pool computed on pre-BN conv output (BN+relu is monotone
    for gamma>0), so the (1024, 2048) x3 tensor is never materialized and
    the g-channels of the c0 conv collapse to a per-core rank-1 bias
    v = Wg @ g folded into the BN shift.
  * repulsion needs each point's 5 smallest pred-pred distances: the PE
    emits -d2s and DVE Max8 yields the 8 largest per row in one pass.
"""
import sys

sys.path.insert(0, "/opt/trn_rl_repo")

import numpy as np

import concourse.bass as bass
import concourse.bacc as bacc
import concourse.tile as tile
import concourse.mybir as mybir

F32 = mybir.dt.float32
F32R = mybir.dt.float32r
AF = mybir.ActivationFunctionType
ALU = mybir.AluOpType
AXL = mybir.AxisListType

N_CORES = 8
N = 2048
M_TOT = float(N_CORES * N)
BN_EPS = 1e-5
REP_H, REP_EPS = 0.01, 1e-12

# (name, Cin, Cout, mode)
LAYERS = [
    ("p0", 3, 64, "hold"),
    ("p1", 64, 128, "hold"),
    ("p2", 128, 1024, "maxpool"),
    ("c0", 64, 512, "copy"),
    ("c1", 512, 256, "copy"),
    ("c2", 256, 128, "hold"),
]


def _sl(i, sz=128):
    return slice(i * sz, (i + 1) * sz)


def param_layout():
    cols, c = {}, 0
    for nm, _, co, _ in LAYERS:
        mts = (co + 127) // 128
        for pfx in ("b", "g", "be"):
            for mt in range(mts):
                cols[(pfx, nm, mt)] = c
                c += 1
    cols[("b", "c3", 0)] = c
    c += 1
    cols[("mask", "h", 0)] = c  # 1.0 on partition row 5, else 0
    c += 1
    return cols, c


PCOLS, NPC = param_layout()


def weight_layout():
    offs, c = {}, 0
    for nm, ci, co, _ in LAYERS:
        kch = (ci + 127) // 128
        offs[nm] = c
        c += kch * co
    offs["c0g"] = c
    c += 8 * 512
    offs["c3"] = c
    c += 3
    return offs, c


WOFF, WTOT = weight_layout()

# layer-stats tile column map (tile shape (128, 128))
SSQ, SYR, GMX, SV, TV, CV, T2C, VAR, MEAN, SX, ST2, FLD = (
    0, 8, 16, 24, 32, 40, 48, 56, 64, 72, 80, 96)

# small_pack column map (tile shape (128, 640))
MINA, MINB, M8C, DA, DB, KNN, KND, HIN, PART, HMX = (
    0, 16, 32, 160, 176, 192, 272, 352, 432, 440)
QC, SUM8, MAX8, TOT8, FIN, MSEV, MSEP, L1A, L1B, SPC = (
    448, 576, 577, 578, 586, 594, 595, 596, 597, 598)


def build_program(dbg=False):
    nc = bacc.Bacc("TRN2", target_bir_lowering=False, debug=False,
                   num_devices=N_CORES)

    pcT_d = nc.dram_tensor("pcT", [3, N], F32, kind="ExternalInput")
    coT_d = nc.dram_tensor("coordT", [3, N], F32, kind="ExternalInput")
    w_all_d = nc.dram_tensor("w_all", [128, WTOT], F32, kind="ExternalInput")
    par_d = nc.dram_tensor("params", [128, NPC], F32, kind="ExternalInput")

    losses_d = nc.dram_tensor("losses", [1, 8], F32, kind="ExternalOutput")
    if dbg:
        pred_d = nc.dram_tensor("pred_dbg", [3, N], F32, kind="ExternalOutput")
        minA_d = nc.dram_tensor("minA_dbg", [128, 16], F32, kind="ExternalOutput")
        minB_d = nc.dram_tensor("minB_dbg", [128, 16], F32, kind="ExternalOutput")
        m8_d = nc.dram_tensor("m8_dbg", [128, 128], F32, kind="ExternalOutput")
        part_d = nc.dram_tensor("part_dbg", [128, 8], F32, kind="ExternalOutput")

    with tile.TileContext(nc) as tc:
        import contextlib
        with contextlib.ExitStack() as ctx:
            io = ctx.enter_context(tc.tile_pool(name="io", bufs=1))
            yst = ctx.enter_context(tc.tile_pool(name="yst", bufs=6))
            aug = ctx.enter_context(tc.tile_pool(name="aug", bufs=1))
            lst = ctx.enter_context(tc.tile_pool(name="lst", bufs=2))
            scr = ctx.enter_context(tc.tile_pool(name="scr", bufs=3))
            scrr = ctx.enter_context(tc.tile_pool(name="scrr", bufs=2))
            psA = ctx.enter_context(tc.tile_pool(name="psA", bufs=2, space="PSUM"))
            dram = ctx.enter_context(tc.tile_pool(name="dram", bufs=1, space="DRAM"))

            # ---------------- load inputs ----------------
            pcT = io.tile([3, N], F32)
            coT = io.tile([3, N], F32)
            nc.sync.dma_start(out=pcT[:], in_=pcT_d[:])
            nc.sync.dma_start(out=coT[:], in_=coT_d[:])
            WA = io.tile([128, WTOT], F32)
            nc.sync.dma_start(out=WA[:], in_=w_all_d[:])
            PAR = io.tile([128, NPC], F32)
            nc.sync.dma_start(out=PAR[:], in_=par_d[:])

            def pv(pfx, nm, mt, mp):
                c = PCOLS[(pfx, nm, mt)]
                return PAR[:mp, c:c + 1]

            CN = io.tile([128, 8], F32)
            for col, val in [(0, 1.0), (1, BN_EPS), (2, 1e-24), (3, REP_EPS),
                             (4, REP_H), (5, 0.0), (6, -1.0)]:
                nc.vector.memset(CN[:, col:col + 1], val)

            SP = io.tile([128, 640], F32)  # small pack

            def spc(c0_, w=1, p=128):
                return SP[:p, c0_:c0_ + w]

            # f32r packs for the distance matmuls:
            #  R1: A_lhsT rows 0-12, B_lhsT rows 32-44
            #  R2: A_rhs  rows 0-12, B_rhs  rows 32-44
            #  R3: C_rhs  rows 0-12
            R1 = aug.tile([128, N], F32R)
            R2 = aug.tile([128, N], F32R)
            R3 = aug.tile([13, N], F32R)
            pred = aug.tile([3, N], F32)

            # ---------------- coord-side distance prep ----------------
            # A_rhs  = [c_hi;c_hi;c_lo;1;1;sc_hi;sc_lo]
            # B_lhsT = [-2c_hi;-2c_lo;-2c_hi;sc_hi;sc_lo;1;1]
            nc.vector.tensor_copy(out=R2[0:3, :], in_=coT[:])          # c_hi
            s_clo = scrr.tile([128, N], F32R, tag="bigr")
            nc.vector.tensor_sub(s_clo[0:3, :], coT[:], R2[0:3, :])    # c_lo
            nc.sync.dma_start(out=R2[3:6, :], in_=R2[0:3, :])
            nc.sync.dma_start(out=R2[6:9, :], in_=s_clo[0:3, :])
            nc.scalar.activation(out=R1[32:35, :], in_=R2[0:3, :],
                                 func=AF.Copy, scale=-2.0)             # -2c_hi
            s_m2clo = scrr.tile([128, N], F32R, tag="bigr")
            nc.scalar.activation(out=s_m2clo[0:3, :], in_=s_clo[0:3, :],
                                 func=AF.Copy, scale=-2.0)
            nc.sync.dma_start(out=R1[35:38, :], in_=s_m2clo[0:3, :])
            nc.sync.dma_start(out=R1[38:41, :], in_=R1[32:35, :])
            # ones row staged once, fanned out by DMA
            s_one = scr.tile([128, N], F32, tag="big")
            nc.vector.memset(s_one[0:1, :], 1.0)
            s_oner = scrr.tile([128, N], F32R, tag="bigr")
            nc.vector.tensor_copy(out=s_oner[0:1, :], in_=s_one[0:1, :])
            for dst, row in ((R2, 9), (R2, 10), (R1, 43), (R1, 44),
                             (R1, 11), (R1, 12), (R2, 41), (R2, 42)):
                nc.sync.dma_start(out=dst[row:row + 1, :], in_=s_oner[0:1, :])
            # sc = sum(coord^2) (fp32) via ones3 matmul
            s_sq = scr.tile([128, N], F32, tag="big")
            nc.scalar.activation(out=s_sq[0:3, :], in_=coT[:], func=AF.Square)
            psc = psA.tile([128, N], F32, tag="big")
            for j in range(4):
                nc.tensor.matmul(psc[0:1, _sl(j, 512)], CN[0:3, 0:1],
                                 s_sq[0:3, _sl(j, 512)], start=True, stop=True)
            s_sc = scr.tile([128, N], F32, tag="big")
            nc.scalar.copy(out=s_sc[0:1, :], in_=psc[0:1, :])
            s_schi = scrr.tile([128, N], F32R, tag="bigr")
            nc.vector.tensor_copy(out=s_schi[0:1, :], in_=s_sc[0:1, :])
            s_sclo = scrr.tile([128, N], F32R, tag="bigr")
            nc.vector.tensor_sub(s_sclo[0:1, :], s_sc[0:1, :], s_schi[0:1, :])
            nc.sync.dma_start(out=R2[11:12, :], in_=s_schi[0:1, :])
            nc.sync.dma_start(out=R2[12:13, :], in_=s_sclo[0:1, :])
            nc.sync.dma_start(out=R1[41:42, :], in_=s_schi[0:1, :])
            nc.sync.dma_start(out=R1[42:43, :], in_=s_sclo[0:1, :])

            # ---------------- forward ----------------
            nc.vector.tensor_reduce(out=spc(SPC, 1, 3), in_=pcT[:],
                                    axis=AXL.X, op=ALU.add)
            x_cur = [pcT]
            s_cur = [spc(SPC, 1, 3)]
            g_tiles = None
            x1_save = s1_save = None

            for nm, ci, co, mode in LAYERS:
                kch = (ci + 127) // 128
                mts = (co + 127) // 128
                L = lst.tile([128, 128], F32, tag="L")

                def wsl(kc, mt, mp, _nm=nm, _co=co, _ci=ci):
                    c = WOFF[_nm] + kc * _co + mt * 128
                    return WA[:min(128, _ci - kc * 128), c:c + mp]

                # Sum-y via W @ s_x (transient psum tile, before conv rotation)
                if mode != "copy":
                    ps1 = psA.tile([128, N], F32, tag="big")
                    for mt in range(mts):
                        mp = min(128, co - mt * 128)
                        for kc in range(kch):
                            nc.tensor.matmul(ps1[:mp, mt:mt + 1], wsl(kc, mt, mp),
                                             s_cur[kc], start=(kc == 0),
                                             stop=(kc == kch - 1))
                    nc.scalar.copy(out=L[:, SYR:SYR + mts], in_=ps1[:, 0:mts])
                if nm == "c0":
                    psv = psA.tile([128, N], F32, tag="big")
                    for mt in range(mts):
                        mp = min(128, co - mt * 128)
                        for kc in range(8):
                            c = WOFF["c0g"] + kc * 512 + mt * 128
                            nc.tensor.matmul(psv[:mp, mt:mt + 1],
                                             WA[:, c:c + mp], g_tiles[kc],
                                             start=(kc == 0), stop=(kc == 7))
                    nc.scalar.copy(out=L[:, CV:CV + mts], in_=psv[:, 0:mts])
                    for mt in range(mts):
                        mp = min(128, co - mt * 128)
                        nc.vector.tensor_add(L[:mp, CV + mt:CV + mt + 1],
                                             L[:mp, CV + mt:CV + mt + 1],
                                             pv("b", nm, mt, mp))

                y_ps, y_sb = [], []
                for mt in range(mts):
                    mp = min(128, co - mt * 128)
                    ps = psA.tile([128, N], F32, tag="big")
                    for kc in range(kch):
                        kr = min(128, ci - kc * 128)
                        for j in range(4):
                            nc.tensor.matmul(ps[:mp, _sl(j, 512)], wsl(kc, mt, mp),
                                             x_cur[kc][:kr, _sl(j, 512)],
                                             start=(kc == 0), stop=(kc == kch - 1))
                    sq_t = scr.tile([128, N], F32, tag="big")
                    nc.scalar.activation(out=sq_t[:mp, :], in_=ps[:mp, :],
                                         func=AF.Square,
                                         accum_out=L[:mp, SSQ + mt:SSQ + mt + 1])
                    if mode == "copy":
                        ysb = yst.tile([128, N], F32, tag="y")
                        nc.scalar.activation(out=ysb[:mp, :], in_=ps[:mp, :],
                                             func=AF.Copy,
                                             accum_out=L[:mp, SYR + mt:SYR + mt + 1])
                        y_sb.append(ysb)
                    else:
                        y_ps.append(ps)
                    if mode == "maxpool":
                        nc.vector.tensor_reduce(out=L[:mp, GMX + mt:GMX + mt + 1],
                                                in_=ps[:mp, :], axis=AXL.X,
                                                op=ALU.max)
                        y_ps.pop()

                # corrected stats: sy += N*c ; ssq += 2c*syr + N*c^2
                for mt in range(mts):
                    mp = min(128, co - mt * 128)
                    cv = (L[:mp, CV + mt:CV + mt + 1] if nm == "c0"
                          else pv("b", nm, mt, mp))
                    st = L[:mp, ST2 + 2 * mt:ST2 + 2 * mt + 2]
                    nc.vector.scalar_tensor_tensor(
                        out=st[:, 0:1], in0=cv, scalar=float(N),
                        in1=L[:mp, SYR + mt:SYR + mt + 1], op0=ALU.mult,
                        op1=ALU.add)
                    nc.vector.scalar_tensor_tensor(
                        out=L[:mp, T2C + mt:T2C + mt + 1], in0=cv, scalar=2.0,
                        in1=L[:mp, SYR + mt:SYR + mt + 1], op0=ALU.mult,
                        op1=ALU.mult)
                    nc.vector.tensor_add(L[:mp, T2C + mt:T2C + mt + 1],
                                         L[:mp, T2C + mt:T2C + mt + 1],
                                         L[:mp, SSQ + mt:SSQ + mt + 1])
                    nc.vector.tensor_scalar(
                        out=st[:, 1:2], in0=cv, scalar1=cv, scalar2=float(N),
                        op0=ALU.mult, op1=ALU.mult)
                    nc.vector.tensor_add(st[:, 1:2], st[:, 1:2],
                                         L[:mp, T2C + mt:T2C + mt + 1])

                # AllGather [sum, sumsq] across cores
                cc_in = dram.tile([co, 2], F32, tag=f"ccin_{nm}")
                cc_out = dram.tile([N_CORES * co, 2], F32, tag=f"ccout_{nm}")
                for mt in range(mts):
                    mp = min(128, co - mt * 128)
                    nc.sync.dma_start(out=cc_in[mt * 128:mt * 128 + mp, :],
                                      in_=L[:mp, ST2 + 2 * mt:ST2 + 2 * mt + 2])
                nc.gpsimd.collective_compute(
                    "AllGather", ALU.bypass,
                    replica_groups=[list(range(N_CORES))],
                    ins=[cc_in[:].opt()], outs=[cc_out[:].opt()])
                base = cc_out[:]
                for mt in range(mts):
                    mp = min(128, co - mt * 128)
                    fold = L[:mp, FLD:FLD + 16].rearrange("p (a b) -> p a b",
                                                          b=N_CORES)
                    ap = bass.AP(tensor=base.tensor,
                                 offset=base.offset + mt * 256,
                                 ap=[[2, mp], [1, 2], [2 * co, N_CORES]])
                    nc.sync.dma_start(out=fold, in_=ap)
                    tot = L[:mp, ST2 + 2 * mt:ST2 + 2 * mt + 2]
                    nc.vector.tensor_reduce(out=tot, in_=fold, axis=AXL.X,
                                            op=ALU.add)
                    mean = L[:mp, MEAN + mt:MEAN + mt + 1]
                    nc.scalar.activation(out=mean, in_=tot[:, 0:1],
                                         func=AF.Copy, scale=1.0 / M_TOT)
                    var = L[:mp, VAR + mt:VAR + mt + 1]
                    nc.vector.tensor_scalar(
                        out=var, in0=mean, scalar1=mean, scalar2=-1.0,
                        op0=ALU.mult, op1=ALU.mult)
                    nc.vector.scalar_tensor_tensor(
                        out=var, in0=tot[:, 1:2], scalar=1.0 / M_TOT,
                        in1=var, op0=ALU.mult, op1=ALU.add)
                    sv = L[:mp, SV + mt:SV + mt + 1]
                    nc.scalar.activation(out=sv, in_=var, func=AF.Sqrt,
                                         bias=CN[:mp, 1:2])
                    nc.vector.reciprocal(out=sv, in_=sv)
                    nc.vector.tensor_mul(sv, pv("g", nm, mt, mp), sv)
                    tv = L[:mp, TV + mt:TV + mt + 1]
                    cv = (L[:mp, CV + mt:CV + mt + 1] if nm == "c0"
                          else pv("b", nm, mt, mp))
                    nc.vector.tensor_sub(tv, cv, mean)
                    nc.vector.tensor_mul(tv, tv, sv)
                    nc.vector.tensor_add(tv, tv, pv("be", nm, mt, mp))

                # apply
                if mode == "maxpool":
                    g_tiles = []
                    for mt in range(mts):
                        gt = L[:, GMX + mt:GMX + mt + 1]
                        nc.scalar.activation(out=gt, in_=gt, func=AF.Relu,
                                             scale=L[:, SV + mt:SV + mt + 1],
                                             bias=L[:, TV + mt:TV + mt + 1])
                        g_tiles.append(gt)
                    x_cur, s_cur = x1_save, s1_save
                else:
                    nx, ns = [], []
                    for mt in range(mts):
                        mp = min(128, co - mt * 128)
                        if mode == "copy":
                            xt = y_sb[mt]
                            src = xt[:mp, :]
                        else:
                            xt = yst.tile([128, N], F32, tag="y")
                            src = y_ps[mt][:mp, :]
                        sx = L[:mp, SX + mt:SX + mt + 1]
                        nc.scalar.activation(out=xt[:mp, :], in_=src,
                                             func=AF.Relu,
                                             scale=L[:mp, SV + mt:SV + mt + 1],
                                             bias=L[:mp, TV + mt:TV + mt + 1],
                                             accum_out=sx)
                        nx.append(xt)
                        ns.append(sx)
                    x_cur, s_cur = nx, ns
                    if nm == "p0":
                        x1_save, s1_save = nx, ns

            # ---------------- c3 -> pred ----------------
            psp = psA.tile([128, N], F32, tag="big")
            c3c = WOFF["c3"]
            for j in range(4):
                nc.tensor.matmul(psp[:3, _sl(j, 512)], WA[:, c3c:c3c + 3],
                                 x_cur[0][:128, _sl(j, 512)], start=True, stop=True)
            nc.scalar.activation(out=pred[:], in_=psp[:3, :], func=AF.Identity,
                                 bias=pv("b", "c3", 0, 3))
            if dbg:
                nc.sync.dma_start(out=pred_d[:], in_=pred[:])

            # ---------------- pred-side distance prep ----------------
            # A_lhsT = [-2p_hi;-2p_lo;-2p_hi;sp_hi;sp_lo;1;1] @ R1 rows 0-12
            # B_rhs  = [p_hi;p_hi;p_lo;1;1;sp_hi;sp_lo]       @ R2 rows 32-44
            s_phi = scrr.tile([128, N], F32R, tag="bigr")
            nc.vector.tensor_copy(out=s_phi[0:3, :], in_=pred[:])      # p_hi
            nc.vector.tensor_copy(out=R2[32:35, :], in_=s_phi[0:3, :])
            nc.sync.dma_start(out=R2[35:38, :], in_=R2[32:35, :])
            s_plo = scrr.tile([128, N], F32R, tag="bigr")
            nc.vector.tensor_sub(s_plo[0:3, :], pred[:], s_phi[0:3, :])
            nc.sync.dma_start(out=R2[38:41, :], in_=s_plo[0:3, :])
            nc.scalar.activation(out=R1[0:3, :], in_=s_phi[0:3, :],
                                 func=AF.Copy, scale=-2.0)             # -2p_hi
            s_m2plo = scrr.tile([128, N], F32R, tag="bigr")
            nc.scalar.activation(out=s_m2plo[0:3, :], in_=s_plo[0:3, :],
                                 func=AF.Copy, scale=-2.0)
            nc.sync.dma_start(out=R1[3:6, :], in_=s_m2plo[0:3, :])
            nc.sync.dma_start(out=R1[6:9, :], in_=R1[0:3, :])
            # sp
            s_sqp = scr.tile([128, N], F32, tag="big")
            nc.scalar.activation(out=s_sqp[0:3, :], in_=pred[:], func=AF.Square)
            psq = psA.tile([128, N], F32, tag="big")
            for j in range(4):
                nc.tensor.matmul(psq[0:1, _sl(j, 512)], CN[0:3, 0:1],
                                 s_sqp[0:3, _sl(j, 512)], start=True, stop=True)
            s_sp = scr.tile([128, N], F32, tag="big")
            nc.scalar.copy(out=s_sp[0:1, :], in_=psq[0:1, :])
            s_sphi = scrr.tile([128, N], F32R, tag="bigr")
            nc.vector.tensor_copy(out=s_sphi[0:1, :], in_=s_sp[0:1, :])
            s_splo = scrr.tile([128, N], F32R, tag="bigr")
            nc.vector.tensor_sub(s_splo[0:1, :], s_sp[0:1, :], s_sphi[0:1, :])
            nc.sync.dma_start(out=R1[9:10, :], in_=s_sphi[0:1, :])
            nc.sync.dma_start(out=R1[10:11, :], in_=s_splo[0:1, :])
            nc.sync.dma_start(out=R2[43:44, :], in_=s_sphi[0:1, :])
            nc.sync.dma_start(out=R2[44:45, :], in_=s_splo[0:1, :])
            # C_rhs = -B_rhs
            nc.vector.tensor_scalar_mul(R3[0:13, :], R2[32:45, :], -1.0)

            # mse / smooth-l1 partials
            s_df = scr.tile([128, N], F32, tag="big")
            nc.vector.tensor_sub(s_df[0:3, :], pred[:], coT[:])
            s_t = scr.tile([128, N], F32, tag="big")
            nc.vector.scalar_tensor_tensor(
                out=s_t[0:3, :], in0=s_df[0:3, :], scalar=1.0, in1=s_df[0:3, :],
                op0=ALU.mult, op1=ALU.mult, accum_out=spc(MSEP, 1, 3))
            s_ab = scr.tile([128, N], F32, tag="big")
            nc.scalar.activation(out=s_ab[0:3, :], in_=s_df[0:3, :], func=AF.Abs)
            s_mn = scr.tile([128, N], F32, tag="big")
            nc.vector.tensor_scalar_min(s_mn[0:3, :], s_ab[0:3, :], 1.0)
            s_u = scr.tile([128, N], F32, tag="big")
            nc.vector.scalar_tensor_tensor(
                out=s_u[0:3, :], in0=s_mn[0:3, :], scalar=0.5, in1=s_mn[0:3, :],
                op0=ALU.mult, op1=ALU.mult, accum_out=spc(L1A, 1, 3))
            s_v2 = scr.tile([128, N], F32, tag="big")
            nc.scalar.activation(out=s_v2[0:3, :], in_=s_ab[0:3, :],
                                 func=AF.Relu, bias=CN[0:3, 6:7],
                                 accum_out=spc(L1B, 1, 3))
            nc.vector.tensor_add(spc(L1A, 1, 3), spc(L1A, 1, 3), spc(L1B, 1, 3))

            # ---------------- distance matrices ----------------
            for nt in range(16):
                pa = psA.tile([128, N], F32, tag="big")
                for j in range(4):
                    nc.tensor.matmul(pa[:, _sl(j, 512)], R1[0:13, _sl(nt)],
                                     R2[0:13, _sl(j, 512)], start=True, stop=True)
                nc.vector.tensor_reduce(out=SP[:, MINA + nt:MINA + nt + 1],
                                        in_=pa[:, :], axis=AXL.X, op=ALU.min)
                pb = psA.tile([128, N], F32, tag="big")
                for j in range(4):
                    nc.tensor.matmul(pb[:, _sl(j, 512)], R1[32:45, _sl(nt)],
                                     R2[32:45, _sl(j, 512)], start=True, stop=True)
                nc.vector.tensor_reduce(out=SP[:, MINB + nt:MINB + nt + 1],
                                        in_=pb[:, :], axis=AXL.X, op=ALU.min)
                pc2 = psA.tile([128, N], F32, tag="big")
                for j in range(4):
                    nc.tensor.matmul(pc2[:, _sl(j, 512)], R1[0:13, _sl(nt)],
                                     R3[0:13, _sl(j, 512)], start=True, stop=True)
                nds = scr.tile([128, N], F32, tag="big")
                nc.scalar.copy(out=nds[:, :], in_=pc2[:, :])
                nc.vector.max(out=SP[:, M8C + 8 * nt:M8C + 8 * nt + 8],
                              in_=nds[:, :])

            if dbg:
                nc.sync.dma_start(out=minA_d[:], in_=spc(MINA, 16))
                nc.sync.dma_start(out=minB_d[:], in_=spc(MINB, 16))
                nc.sync.dma_start(out=m8_d[:], in_=spc(M8C, 128))

            # ---------------- per-core loss partials ----------------
            nc.vector.tensor_scalar_max(spc(MINA, 16), spc(MINA, 16), 0.0)
            nc.vector.tensor_scalar_max(spc(MINB, 16), spc(MINB, 16), 0.0)
            nc.scalar.activation(out=spc(DA, 16), in_=spc(MINA, 16),
                                 func=AF.Sqrt, bias=CN[:, 2:3])
            nc.scalar.activation(out=spc(DB, 16), in_=spc(MINB, 16),
                                 func=AF.Sqrt, bias=CN[:, 2:3])
            nc.vector.memset(spc(PART, 8), 0.0)
            nc.vector.tensor_reduce(out=spc(PART + 0), in_=spc(DA, 16),
                                    axis=AXL.X, op=ALU.add)
            nc.vector.tensor_reduce(out=spc(PART + 1), in_=spc(DB, 16),
                                    axis=AXL.X, op=ALU.add)
            nc.vector.tensor_reduce(out=spc(HMX + 0), in_=spc(DA, 16),
                                    axis=AXL.X, op=ALU.max)
            nc.vector.tensor_reduce(out=spc(HMX + 1), in_=spc(DB, 16),
                                    axis=AXL.X, op=ALU.max)
            nc.vector.tensor_reduce(out=spc(PART + 5), in_=spc(HMX, 2),
                                    axis=AXL.X, op=ALU.max)
            # repulsion hinge on knn slots 1..5 of each row
            m8v = spc(M8C, 128).rearrange("p (a b) -> p a b", b=8)
            nc.vector.tensor_scalar(
                out=spc(KNN, 80).rearrange("p (a b) -> p a b", b=5),
                in0=m8v[:, :, 1:6], scalar1=-1.0, scalar2=0.0,
                op0=ALU.mult, op1=ALU.max)
            nc.scalar.activation(out=spc(KND, 80), in_=spc(KNN, 80),
                                 func=AF.Sqrt, bias=CN[:, 3:4])
            nc.scalar.activation(out=spc(HIN, 80), in_=spc(KND, 80),
                                 func=AF.Relu, scale=-1.0, bias=CN[:, 4:5])
            nc.vector.tensor_reduce(out=spc(PART + 2), in_=spc(HIN, 80),
                                    axis=AXL.X, op=ALU.add)
            nc.vector.tensor_copy(out=spc(PART + 3, 1, 3), in_=spc(MSEP, 1, 3))
            nc.vector.tensor_copy(out=spc(PART + 4, 1, 3), in_=spc(L1A, 1, 3))
            if dbg:
                nc.sync.dma_start(out=part_d[:], in_=spc(PART, 8))

            # fold partitions via DRAM bounce transpose
            pp = dram.tile([128, 8], F32)
            nc.sync.dma_start(out=pp[:], in_=spc(PART, 8))
            ppb = pp[:]
            nc.sync.dma_start(out=SP[0:8, QC:QC + 128],
                              in_=bass.AP(tensor=ppb.tensor, offset=ppb.offset,
                                          ap=[[1, 8], [8, 128]]))
            nc.vector.tensor_reduce(out=spc(SUM8, 1, 8), in_=SP[0:8, QC:QC + 128],
                                    axis=AXL.X, op=ALU.add)
            nc.vector.tensor_reduce(out=spc(MAX8, 1, 8), in_=SP[0:8, QC:QC + 128],
                                    axis=AXL.X, op=ALU.max)
            # row 5 (per-core hausdorff max) comes from the max-reduce:
            # merged = sums + mask * (maxs - sums), mask = 1 only on row 5
            hm = PCOLS[("mask", "h", 0)]
            nc.vector.tensor_sub(spc(MAX8, 1, 8), spc(MAX8, 1, 8),
                                 spc(SUM8, 1, 8))
            nc.vector.tensor_mul(spc(MAX8, 1, 8), spc(MAX8, 1, 8),
                                 PAR[:8, hm:hm + 1])
            nc.vector.tensor_add(spc(SUM8, 1, 8), spc(SUM8, 1, 8),
                                 spc(MAX8, 1, 8))

            ar_in = dram.tile([8, 1], F32)
            ar_out = dram.tile([8, 1], F32)
            nc.sync.dma_start(out=ar_in[:], in_=spc(SUM8, 1, 8))
            nc.gpsimd.collective_compute(
                "AllReduce", ALU.add, replica_groups=[list(range(N_CORES))],
                ins=[ar_in[:].opt()], outs=[ar_out[:].opt()])
            arb = ar_out[:]
            nc.sync.dma_start(out=SP[0:1, TOT8:TOT8 + 8],
                              in_=bass.AP(tensor=arb.tensor, offset=arb.offset,
                                          ap=[[8, 1], [1, 8]]))

            def T8(i):
                return SP[0:1, TOT8 + i:TOT8 + i + 1]

            def FN(i):
                return SP[0:1, FIN + i:FIN + i + 1]

            nc.vector.memset(SP[0:1, FIN:FIN + 8], 0.0)
            nc.vector.tensor_add(FN(1), T8(0), T8(1))
            nc.scalar.activation(out=FN(1), in_=FN(1), func=AF.Copy,
                                 scale=1.0 / 16384.0)
            nc.scalar.activation(out=FN(2), in_=T8(2), func=AF.Copy,
                                 scale=1.0 / 81920.0)
            nc.scalar.activation(out=SP[0:1, MSEV:MSEV + 1], in_=T8(3),
                                 func=AF.Copy, scale=1.0 / 49152.0)
            nc.scalar.activation(out=FN(3), in_=SP[0:1, MSEV:MSEV + 1],
                                 func=AF.Sqrt, bias=CN[0:1, 5:6])
            nc.scalar.activation(out=FN(0), in_=FN(3), func=AF.Copy, scale=10.0)
            nc.scalar.activation(out=FN(4), in_=T8(5), func=AF.Copy,
                                 scale=1.0 / 8.0)
            nc.scalar.activation(out=FN(5), in_=T8(4), func=AF.Copy,
                                 scale=1.0 / 49152.0)
            nc.sync.dma_start(out=losses_d[:], in_=SP[0:1, FIN:FIN + 8])

    nc.compile()
    return nc


_PROG_CACHE = {}


def _get_prog(dbg=False):
    if dbg not in _PROG_CACHE:
        _PROG_CACHE[dbg] = build_program(dbg)
    return _PROG_CACHE[dbg]


def make_in_maps(pc, coord, params):
    pc = np.asarray(pc, dtype=np.float32)
    coord = np.asarray(coord, dtype=np.float32)
    pr = {k: np.asarray(v, dtype=np.float32) for k, v in params.items()}

    w_all = np.zeros((128, WTOT), dtype=np.float32)

    def wput(nm, wt, ci, co):
        kch = (ci + 127) // 128
        for kc in range(kch):
            r = min(128, ci - kc * 128)
            w_all[:r, WOFF[nm] + kc * co:WOFF[nm] + (kc + 1) * co] = \
                wt[kc * 128:kc * 128 + r, :]

    wput("p0", pr["pW0"].T, 3, 64)
    wput("p1", pr["pW1"].T, 64, 128)
    wput("p2", pr["pW2"].T, 128, 1024)
    wput("c0", pr["cW0"][:, :64].T, 64, 512)
    wput("c0g", pr["cW0"][:, 64:].T, 1024, 512)
    wput("c1", pr["cW1"].T, 512, 256)
    wput("c2", pr["cW2"].T, 256, 128)
    wput("c3", pr["cW3"].T, 128, 3)

    par = np.zeros((128, NPC), dtype=np.float32)
    src = {"b": {"p": "pb", "c": "cb"}, "g": {"p": "pg", "c": "cg"},
           "be": {"p": "pbeta", "c": "cbeta"}}
    for nm, _, co, _ in LAYERS:
        mts = (co + 127) // 128
        for pfx in ("b", "g", "be"):
            vec = pr[src[pfx][nm[0]] + nm[1]]
            for mt in range(mts):
                mp = min(128, co - mt * 128)
                par[:mp, PCOLS[(pfx, nm, mt)]] = vec[mt * 128:mt * 128 + mp]
    par[:3, PCOLS[("b", "c3", 0)]] = pr["cb3"]
    par[5, PCOLS[("mask", "h", 0)]] = 1.0

    in_maps = []
    for b in range(N_CORES):
        m = {"w_all": w_all, "params": par,
             "pcT": np.ascontiguousarray(pc[b].T),
             "coordT": np.ascontiguousarray(coord[b].T)}
        in_maps.append(m)
    return in_maps


def kernel(pc, coord, params):
    from concourse.bass_utils import run_bass_kernel_spmd
    nc = _get_prog(dbg=False)
    in_maps = make_in_maps(pc, coord, params)
    res = run_bass_kernel_spmd(nc, in_maps, core_ids=list(range(N_CORES)))
    out = np.asarray(res.results[0]["losses"]).reshape(-1)[:6].astype(np.float32)
    return out


# revision 28
# speedup vs baseline: 1.6836x; 1.6836x over previous
"""Trainium2 Bass kernel for nn_EstCoordNet (PointNet-style loss_fn).

Sharding: data-parallel over B=8 across 8 NeuronCores (one batch item per
core).  BN batch stats are combined with one small AllGather per BN layer;
the final scalar losses with one AllReduce.  Convs run as plain fp32
matmuls; the three 2048x2048 distance matrices run as error-compensated
float32r matmuls (hi/lo split -> ~fp32 accuracy at full PE rate).

Key structural tricks:
  * activations kept channel-major (C, 2048) so BN scale/bias are
    per-partition vectors and one fused ACT Relu applies BN+relu.
  * global max-pool computed on pre-BN conv output (BN+relu is monotone
    for gamma>0), so the (1024, 2048) x3 tensor is never materialized and
    the g-channels of the c0 conv collapse to a per-core rank-1 bias
    v = Wg @ g folded into the BN shift.
  * repulsion needs each point's 5 smallest pred-pred distances: the PE
    emits -d2s and DVE Max8 yields the 8 largest per row in one pass.
"""
import sys

sys.path.insert(0, "/opt/trn_rl_repo")

import numpy as np

import concourse.bass as bass
import concourse.bacc as bacc
import concourse.tile as tile
import concourse.mybir as mybir

F32 = mybir.dt.float32
F32R = mybir.dt.float32r
AF = mybir.ActivationFunctionType
ALU = mybir.AluOpType
AXL = mybir.AxisListType

N_CORES = 8
N = 2048
M_TOT = float(N_CORES * N)
BN_EPS = 1e-5
REP_H, REP_EPS = 0.01, 1e-12

# (name, Cin, Cout, mode)
LAYERS = [
    ("p0", 3, 64, "hold"),
    ("p1", 64, 128, "hold"),
    ("p2", 128, 1024, "maxpool"),
    ("c0", 64, 512, "copy"),
    ("c1", 512, 256, "copy"),
    ("c2", 256, 128, "hold"),
]


def _sl(i, sz=128):
    return slice(i * sz, (i + 1) * sz)


def param_layout():
    cols, c = {}, 0
    for nm, _, co, _ in LAYERS:
        mts = (co + 127) // 128
        for pfx in ("b", "g", "be"):
            for mt in range(mts):
                cols[(pfx, nm, mt)] = c
                c += 1
    cols[("b", "c3", 0)] = c
    c += 1
    cols[("mask", "h", 0)] = c  # 1.0 on partition row 5, else 0
    c += 1
    return cols, c


PCOLS, NPC = param_layout()


def weight_layout():
    offs, c = {}, 0
    for nm, ci, co, _ in LAYERS:
        kch = (ci + 127) // 128
        offs[nm] = c
        c += kch * co
    offs["c0g"] = c
    c += 8 * 512
    offs["c3"] = c
    c += 3
    return offs, c


WOFF, WTOT = weight_layout()

# layer-stats tile column map (tile shape (128, 224))
SSQ, SYR, GMX, SV, TV, CV, T2C, VAR, MEAN, SX, ST2S, ST2Q, FLD, FLDQ = (
    0, 8, 16, 24, 32, 40, 48, 56, 64, 72, 80, 88, 96, 160)

# small_pack column map (tile shape (128, 640))
MINA, MINB, M8C, DA, DB, KNN, KND, HIN, PART, HMX = (
    0, 16, 32, 160, 176, 192, 272, 352, 432, 440)
QC, SUM8, MAX8, TOT8, FIN, MSEV, MSEP, L1A, L1B, SPC = (
    448, 576, 577, 578, 586, 594, 595, 596, 597, 598)


class _EarlyExit(Exception):
    pass


def build_program(dbg=False, nocc=False, skip_dist=False, layers_n=None, cut=None):
    nc = bacc.Bacc("TRN2", target_bir_lowering=False, debug=False,
                   num_devices=N_CORES)

    pcT_d = nc.dram_tensor("pcT", [3, N], F32R, kind="ExternalInput")
    coT_d = nc.dram_tensor("coordT", [3, N], F32, kind="ExternalInput")
    w_all_d = nc.dram_tensor("w_all", [128, WTOT], F32R, kind="ExternalInput")
    par_d = nc.dram_tensor("params", [128, NPC], F32, kind="ExternalInput")

    losses_d = nc.dram_tensor("losses", [1, 8], F32, kind="ExternalOutput")
    if dbg:
        pred_d = nc.dram_tensor("pred_dbg", [3, N], F32, kind="ExternalOutput")
        minA_d = nc.dram_tensor("minA_dbg", [128, 16], F32, kind="ExternalOutput")
        minB_d = nc.dram_tensor("minB_dbg", [128, 16], F32, kind="ExternalOutput")
        m8_d = nc.dram_tensor("m8_dbg", [128, 128], F32, kind="ExternalOutput")
        part_d = nc.dram_tensor("part_dbg", [128, 8], F32, kind="ExternalOutput")

    with tile.TileContext(nc) as tc:
      def _emit():
        import contextlib
        with contextlib.ExitStack() as ctx:
            io = ctx.enter_context(tc.tile_pool(name="io", bufs=1))
            yst = ctx.enter_context(tc.tile_pool(name="yst", bufs=6))
            aug = ctx.enter_context(tc.tile_pool(name="aug", bufs=1))
            lst = ctx.enter_context(tc.tile_pool(name="lst", bufs=2))
            scr = ctx.enter_context(tc.tile_pool(name="scr", bufs=3))
            scrr = ctx.enter_context(tc.tile_pool(name="scrr", bufs=2))
            psA = ctx.enter_context(tc.tile_pool(name="psA", bufs=2, space="PSUM"))
            dram = ctx.enter_context(tc.tile_pool(name="dram", bufs=1, space="DRAM"))

            # ---------------- load inputs ----------------
            pcT = io.tile([3, N], F32R)
            coT = io.tile([3, N], F32)
            nc.sync.dma_start(out=pcT[:], in_=pcT_d[:])
            nc.sync.dma_start(out=coT[:], in_=coT_d[:])
            WA = io.tile([128, WTOT], F32R)
            # split the 3.6MB weight load into ~256KB pieces so it spreads
            # across DMA queues and layers only wait for their own slice
            wcuts = []
            for nm, ci, co, _ in LAYERS:
                kch = (ci + 127) // 128
                for kc in range(kch):
                    wcuts.append(WOFF[nm] + kc * co)
            for kc in range(8):
                wcuts.append(WOFF["c0g"] + kc * 512)
            wcuts.append(WOFF["c3"])
            wcuts.append(WTOT)
            wcuts = sorted(set(wcuts))
            for a, b in zip(wcuts[:-1], wcuts[1:]):
                nc.sync.dma_start(out=WA[:, a:b], in_=w_all_d[:, a:b])
            PAR = io.tile([128, NPC], F32)
            nc.sync.dma_start(out=PAR[:], in_=par_d[:])

            def pv(pfx, nm, mt, mp):
                c = PCOLS[(pfx, nm, mt)]
                return PAR[:mp, c:c + 1]

            CN = io.tile([128, 8], F32)
            for col, val in [(0, 1.0), (1, BN_EPS), (2, 1e-24), (3, REP_EPS),
                             (4, REP_H), (5, 0.0), (6, -1.0)]:
                nc.vector.memset(CN[:, col:col + 1], val)

            SP = io.tile([128, 640], F32)  # small pack

            def spc(c0_, w=1, p=128):
                return SP[:p, c0_:c0_ + w]

            # f32r packs for the distance matmuls:
            #  R1: A_lhsT rows 0-12, B_lhsT rows 32-44
            #  R2: A_rhs  rows 0-12, B_rhs  rows 32-44
            #  R3: C_rhs  rows 0-12
            R1 = aug.tile([128, N], F32R)
            R2 = aug.tile([128, N], F32R)
            R3 = aug.tile([13, N], F32R)
            pred = aug.tile([3, N], F32)

            # ---------------- coord-side distance prep ----------------
            # A_rhs  = [c_hi;c_hi;c_lo;1;1;sc_hi;sc_lo]
            # B_lhsT = [-2c_hi;-2c_lo;-2c_hi;sc_hi;sc_lo;1;1]
            nc.vector.tensor_copy(out=R2[0:3, :], in_=coT[:])          # c_hi
            s_clo = scrr.tile([128, N], F32R, tag="bigr")
            nc.vector.tensor_sub(s_clo[0:3, :], coT[:], R2[0:3, :])    # c_lo
            nc.sync.dma_start(out=R2[3:6, :], in_=R2[0:3, :])
            nc.sync.dma_start(out=R2[6:9, :], in_=s_clo[0:3, :])
            nc.scalar.activation(out=R1[32:35, :], in_=R2[0:3, :],
                                 func=AF.Copy, scale=-2.0)             # -2c_hi
            s_m2clo = scrr.tile([128, N], F32R, tag="bigr")
            nc.scalar.activation(out=s_m2clo[0:3, :], in_=s_clo[0:3, :],
                                 func=AF.Copy, scale=-2.0)
            nc.sync.dma_start(out=R1[35:38, :], in_=s_m2clo[0:3, :])
            nc.sync.dma_start(out=R1[38:41, :], in_=R1[32:35, :])
            # ones row staged once, fanned out by DMA
            s_one = scr.tile([128, N], F32, tag="big")
            nc.vector.memset(s_one[0:1, :], 1.0)
            s_oner = scrr.tile([128, N], F32R, tag="bigr")
            nc.vector.tensor_copy(out=s_oner[0:1, :], in_=s_one[0:1, :])
            for dst, row in ((R2, 9), (R2, 10), (R1, 43), (R1, 44),
                             (R1, 11), (R1, 12), (R2, 41), (R2, 42)):
                nc.sync.dma_start(out=dst[row:row + 1, :], in_=s_oner[0:1, :])
            # sc = sum(coord^2) (fp32) via ones3 matmul
            s_sq = scr.tile([128, N], F32, tag="big")
            nc.scalar.activation(out=s_sq[0:3, :], in_=coT[:], func=AF.Square)
            psc = psA.tile([128, N], F32, tag="big")
            for j in range(4):
                nc.tensor.matmul(psc[0:1, _sl(j, 512)], CN[0:3, 0:1],
                                 s_sq[0:3, _sl(j, 512)], start=True, stop=True)
            s_sc = scr.tile([128, N], F32, tag="big")
            nc.scalar.copy(out=s_sc[0:1, :], in_=psc[0:1, :])
            s_schi = scrr.tile([128, N], F32R, tag="bigr")
            nc.vector.tensor_copy(out=s_schi[0:1, :], in_=s_sc[0:1, :])
            s_sclo = scrr.tile([128, N], F32R, tag="bigr")
            nc.vector.tensor_sub(s_sclo[0:1, :], s_sc[0:1, :], s_schi[0:1, :])
            nc.sync.dma_start(out=R2[11:12, :], in_=s_schi[0:1, :])
            nc.sync.dma_start(out=R2[12:13, :], in_=s_sclo[0:1, :])
            nc.sync.dma_start(out=R1[41:42, :], in_=s_schi[0:1, :])
            nc.sync.dma_start(out=R1[42:43, :], in_=s_sclo[0:1, :])

            # ---------------- forward ----------------
            nc.vector.tensor_reduce(out=spc(SPC, 1, 3), in_=pcT[:],
                                    axis=AXL.X, op=ALU.add)
            x_cur = [pcT]
            s_cur = [spc(SPC, 1, 3)]
            g_tiles = None
            x1_save = s1_save = None

            for nm, ci, co, mode in (LAYERS if layers_n is None else LAYERS[:layers_n]):
                kch = (ci + 127) // 128
                mts = (co + 127) // 128
                L = lst.tile([128, 224], F32, tag="L")

                def wsl(kc, mt, mp, _nm=nm, _co=co, _ci=ci):
                    c = WOFF[_nm] + kc * _co + mt * 128
                    return WA[:min(128, _ci - kc * 128), c:c + mp]

                # Sum-y via W @ s_x (transient psum tile, before conv rotation)
                if True:
                    ps1 = psA.tile([128, N], F32, tag="big")
                    for mt in range(mts):
                        mp = min(128, co - mt * 128)
                        for kc in range(kch):
                            nc.tensor.matmul(ps1[:mp, mt:mt + 1],
                                             wsl(kc, mt, mp).bitcast(F32),
                                             s_cur[kc], start=(kc == 0),
                                             stop=(kc == kch - 1))
                    nc.scalar.copy(out=L[:, SYR:SYR + mts], in_=ps1[:, 0:mts])
                if nm == "c0":
                    psv = psA.tile([128, N], F32, tag="big")
                    for mt in range(mts):
                        mp = min(128, co - mt * 128)
                        for kc in range(8):
                            c = WOFF["c0g"] + kc * 512 + mt * 128
                            nc.tensor.matmul(psv[:mp, mt:mt + 1],
                                             WA[:, c:c + mp].bitcast(F32),
                                             g_tiles[kc],
                                             start=(kc == 0), stop=(kc == 7))
                    nc.scalar.copy(out=L[:, CV:CV + mts], in_=psv[:, 0:mts])
                    for mt in range(mts):
                        mp = min(128, co - mt * 128)
                        nc.vector.tensor_add(L[:mp, CV + mt:CV + mt + 1],
                                             L[:mp, CV + mt:CV + mt + 1],
                                             pv("b", nm, mt, mp))

                y_ps, y_sb = [], []
                for mt in range(mts):
                    mp = min(128, co - mt * 128)
                    ps = psA.tile([128, N], F32, tag="big")
                    for kc in range(kch):
                        kr = min(128, ci - kc * 128)
                        for j in range(4):
                            xs = x_cur[kc][:kr, _sl(j, 512)]
                            if xs.dtype != F32R:
                                xs = xs.bitcast(F32R)
                            nc.tensor.matmul(
                                ps[:mp, _sl(j, 512)], wsl(kc, mt, mp), xs,
                                start=(kc == 0), stop=(kc == kch - 1))
                    if mode == "copy":
                        # DVE: psum->sbuf rounded copy; ACT: square+accum from
                        # the copy (parallel engines). Sum-y comes from the
                        # W @ s_x matmul path like the other layers.
                        ysb = yst.tile([128, N], F32, tag="y")
                        nc.vector.tensor_copy(out=ysb[:mp, :].bitcast(F32R),
                                              in_=ps[:mp, :])
                        sq_t = scr.tile([128, N], F32, tag="big")
                        nc.scalar.activation(out=sq_t[:mp, :], in_=ysb[:mp, :],
                                             func=AF.Square,
                                             accum_out=L[:mp, SSQ + mt:SSQ + mt + 1])
                        y_sb.append(ysb)
                    else:
                        sq_t = scr.tile([128, N], F32, tag="big")
                        nc.scalar.activation(out=sq_t[:mp, :], in_=ps[:mp, :],
                                             func=AF.Square,
                                             accum_out=L[:mp, SSQ + mt:SSQ + mt + 1])
                        y_ps.append(ps)
                    if mode == "maxpool":
                        nc.vector.tensor_reduce(out=L[:mp, GMX + mt:GMX + mt + 1],
                                                in_=ps[:mp, :], axis=AXL.X,
                                                op=ALU.max)
                        y_ps.pop()

                # ---- layer-wide (128, mts) stat math ----
                cvw = (L[:, CV:CV + mts] if nm == "c0"
                       else PAR[:, PCOLS[("b", nm, 0)]:PCOLS[("b", nm, 0)] + mts])
                syw = L[:, SYR:SYR + mts]
                ssqw = L[:, SSQ:SSQ + mts]
                sumc = L[:, ST2S:ST2S + mts]
                ssqc = L[:, ST2Q:ST2Q + mts]
                t2c = L[:, T2C:T2C + mts]
                # corrected stats: sy += N*c ; ssq += 2c*sy_raw + N*c^2
                nc.vector.scalar_tensor_tensor(
                    out=sumc, in0=cvw, scalar=float(N), in1=syw,
                    op0=ALU.mult, op1=ALU.add)
                nc.vector.scalar_tensor_tensor(
                    out=t2c, in0=cvw, scalar=2.0, in1=syw,
                    op0=ALU.mult, op1=ALU.mult)
                nc.vector.tensor_add(t2c, t2c, ssqw)
                sqv = L[:, VAR:VAR + mts]
                nc.vector.tensor_mul(sqv, cvw, cvw)
                nc.vector.scalar_tensor_tensor(
                    out=ssqc, in0=sqv, scalar=float(N), in1=t2c,
                    op0=ALU.mult, op1=ALU.add)

                # AllGather [sums(co); ssq(co)] across cores
                cc_in = dram.tile([2, co], F32, tag=f"ccin_{nm}")
                cc_out = dram.tile([2 * N_CORES, co], F32, tag=f"ccout_{nm}")
                cib = cc_in[:]
                for mt in range(mts):
                    mp = min(128, co - mt * 128)
                    nc.sync.dma_start(
                        out=bass.AP(tensor=cib.tensor,
                                    offset=cib.offset + mt * 128,
                                    ap=[[1, mp]]),
                        in_=L[:mp, ST2S + mt:ST2S + mt + 1])
                    nc.sync.dma_start(
                        out=bass.AP(tensor=cib.tensor,
                                    offset=cib.offset + co + mt * 128,
                                    ap=[[1, mp]]),
                        in_=L[:mp, ST2Q + mt:ST2Q + mt + 1])
                if nocc:
                    nc.sync.dma_start(out=cc_out[0:2, :], in_=cc_in[:])
                else:
                    nc.gpsimd.collective_compute(
                        "AllGather", ALU.bypass,
                        replica_groups=[list(range(N_CORES))],
                        ins=[cc_in[:].opt()], outs=[cc_out[:].opt()])
                base = cc_out[:]
                # fold across cores: per-mt DMAs, one wide reduce per block
                for blk, off in ((FLD, 0), (FLDQ, co)):
                    for mt in range(mts):
                        mp = min(128, co - mt * 128)
                        ap = bass.AP(tensor=base.tensor,
                                     offset=base.offset + off + mt * 128,
                                     ap=[[1, mp], [2 * co, N_CORES]])
                        nc.sync.dma_start(
                            out=L[:mp, blk + 8 * mt:blk + 8 * mt + 8], in_=ap)
                    fold = L[:, blk:blk + 8 * mts].rearrange(
                        "p (m b) -> p m b", b=N_CORES)
                    dst = sumc if off == 0 else ssqc
                    nc.vector.tensor_reduce(out=dst, in_=fold, axis=AXL.X,
                                            op=ALU.add)
                meanw = L[:, MEAN:MEAN + mts]
                nc.scalar.activation(out=meanw, in_=sumc, func=AF.Copy,
                                     scale=1.0 / M_TOT)
                nc.vector.scalar_tensor_tensor(
                    out=t2c, in0=meanw, scalar=-1.0, in1=meanw,
                    op0=ALU.mult, op1=ALU.mult)
                varw = L[:, VAR:VAR + mts]
                nc.vector.scalar_tensor_tensor(
                    out=varw, in0=ssqc, scalar=1.0 / M_TOT, in1=t2c,
                    op0=ALU.mult, op1=ALU.add)
                svw = L[:, SV:SV + mts]
                nc.scalar.activation(out=svw, in_=varw, func=AF.Sqrt,
                                     bias=CN[:, 1:2])
                nc.vector.reciprocal(out=svw, in_=svw)
                gcol = PCOLS[("g", nm, 0)]
                nc.vector.tensor_mul(svw, PAR[:, gcol:gcol + mts], svw)
                tvw = L[:, TV:TV + mts]
                nc.vector.tensor_sub(tvw, cvw, meanw)
                nc.vector.tensor_mul(tvw, tvw, svw)
                becol = PCOLS[("be", nm, 0)]
                nc.vector.tensor_add(tvw, tvw, PAR[:, becol:becol + mts])

                # apply
                if mode == "maxpool":
                    gw = L[:, GMX:GMX + mts]
                    nc.vector.tensor_mul(gw, gw, svw)
                    nc.vector.tensor_add(gw, gw, tvw)
                    nc.vector.tensor_scalar_max(gw, gw, 0.0)
                    g_tiles = [L[:, GMX + mt:GMX + mt + 1] for mt in range(mts)]
                    x_cur, s_cur = x1_save, s1_save
                else:
                    nx, ns = [], []
                    for mt in range(mts):
                        mp = min(128, co - mt * 128)
                        if mode == "copy":
                            xt = y_sb[mt]
                            src = xt[:mp, :]
                        else:
                            xt = yst.tile([128, N], F32, tag="y")
                            src = y_ps[mt][:mp, :]
                        sx = L[:mp, SX + mt:SX + mt + 1]
                        xdst = (xt[:mp, :] if nm == "c2"
                                else xt[:mp, :].bitcast(F32R))
                        nc.scalar.activation(out=xdst, in_=src,
                                             func=AF.Relu,
                                             scale=L[:mp, SV + mt:SV + mt + 1],
                                             bias=L[:mp, TV + mt:TV + mt + 1],
                                             accum_out=sx)
                        nx.append(xt)
                        ns.append(sx)
                    x_cur, s_cur = nx, ns
                    if nm == "p0":
                        x1_save, s1_save = nx, ns

            # ---------------- c3 -> pred ----------------
            if layers_n is not None:
                nc.sync.dma_start(out=losses_d[:], in_=CN[0:1, 0:8])
                return
            psp = psA.tile([128, N], F32, tag="big")
            c3c = WOFF["c3"]
            for j in range(4):
                nc.tensor.matmul(psp[:3, _sl(j, 512)],
                                 WA[:, c3c:c3c + 3].bitcast(F32),
                                 x_cur[0][:128, _sl(j, 512)], start=True, stop=True)
            nc.scalar.activation(out=pred[:], in_=psp[:3, :], func=AF.Identity,
                                 bias=pv("b", "c3", 0, 3))
            if dbg:
                nc.sync.dma_start(out=pred_d[:], in_=pred[:])
            if cut == "c3":
                nc.sync.dma_start(out=losses_d[:], in_=CN[0:1, 0:8])
                return

            # ---------------- pred-side distance prep ----------------
            # A_lhsT = [-2p_hi;-2p_lo;-2p_hi;sp_hi;sp_lo;1;1] @ R1 rows 0-12
            # B_rhs  = [p_hi;p_hi;p_lo;1;1;sp_hi;sp_lo]       @ R2 rows 32-44
            s_phi = scrr.tile([128, N], F32R, tag="bigr")
            nc.vector.tensor_copy(out=s_phi[0:3, :], in_=pred[:])      # p_hi
            nc.vector.tensor_copy(out=R2[32:35, :], in_=s_phi[0:3, :])
            nc.sync.dma_start(out=R2[35:38, :], in_=R2[32:35, :])
            s_plo = scrr.tile([128, N], F32R, tag="bigr")
            nc.vector.tensor_sub(s_plo[0:3, :], pred[:], s_phi[0:3, :])
            nc.sync.dma_start(out=R2[38:41, :], in_=s_plo[0:3, :])
            nc.scalar.activation(out=R1[0:3, :], in_=s_phi[0:3, :],
                                 func=AF.Copy, scale=-2.0)             # -2p_hi
            s_m2plo = scrr.tile([128, N], F32R, tag="bigr")
            nc.scalar.activation(out=s_m2plo[0:3, :], in_=s_plo[0:3, :],
                                 func=AF.Copy, scale=-2.0)
            nc.sync.dma_start(out=R1[3:6, :], in_=s_m2plo[0:3, :])
            nc.sync.dma_start(out=R1[6:9, :], in_=R1[0:3, :])
            # sp
            s_sqp = scr.tile([128, N], F32, tag="big")
            nc.scalar.activation(out=s_sqp[0:3, :], in_=pred[:], func=AF.Square)
            psq = psA.tile([128, N], F32, tag="big")
            for j in range(4):
                nc.tensor.matmul(psq[0:1, _sl(j, 512)], CN[0:3, 0:1],
                                 s_sqp[0:3, _sl(j, 512)], start=True, stop=True)
            s_sp = scr.tile([128, N], F32, tag="big")
            nc.scalar.copy(out=s_sp[0:1, :], in_=psq[0:1, :])
            s_sphi = scrr.tile([128, N], F32R, tag="bigr")
            nc.vector.tensor_copy(out=s_sphi[0:1, :], in_=s_sp[0:1, :])
            s_splo = scrr.tile([128, N], F32R, tag="bigr")
            nc.vector.tensor_sub(s_splo[0:1, :], s_sp[0:1, :], s_sphi[0:1, :])
            nc.sync.dma_start(out=R1[9:10, :], in_=s_sphi[0:1, :])
            nc.sync.dma_start(out=R1[10:11, :], in_=s_splo[0:1, :])
            nc.sync.dma_start(out=R2[43:44, :], in_=s_sphi[0:1, :])
            nc.sync.dma_start(out=R2[44:45, :], in_=s_splo[0:1, :])
            # C_rhs = -B_rhs
            nc.vector.tensor_scalar_mul(R3[0:13, :], R2[32:45, :], -1.0)

            if cut == "prep":
                nc.sync.dma_start(out=losses_d[:], in_=CN[0:1, 0:8])
                return
            # mse / smooth-l1 partials
            s_df = scr.tile([128, N], F32, tag="big")
            nc.vector.tensor_sub(s_df[0:3, :], pred[:], coT[:])
            s_t = scr.tile([128, N], F32, tag="big")
            nc.vector.scalar_tensor_tensor(
                out=s_t[0:3, :], in0=s_df[0:3, :], scalar=1.0, in1=s_df[0:3, :],
                op0=ALU.mult, op1=ALU.mult, accum_out=spc(MSEP, 1, 3))
            s_ab = scr.tile([128, N], F32, tag="big")
            nc.scalar.activation(out=s_ab[0:3, :], in_=s_df[0:3, :], func=AF.Abs)
            s_mn = scr.tile([128, N], F32, tag="big")
            nc.vector.tensor_scalar_min(s_mn[0:3, :], s_ab[0:3, :], 1.0)
            s_u = scr.tile([128, N], F32, tag="big")
            nc.vector.scalar_tensor_tensor(
                out=s_u[0:3, :], in0=s_mn[0:3, :], scalar=0.5, in1=s_mn[0:3, :],
                op0=ALU.mult, op1=ALU.mult, accum_out=spc(L1A, 1, 3))
            s_v2 = scr.tile([128, N], F32, tag="big")
            nc.scalar.activation(out=s_v2[0:3, :], in_=s_ab[0:3, :],
                                 func=AF.Relu, bias=CN[0:3, 6:7],
                                 accum_out=spc(L1B, 1, 3))
            nc.vector.tensor_add(spc(L1A, 1, 3), spc(L1A, 1, 3), spc(L1B, 1, 3))

            if cut == "mse":
                nc.sync.dma_start(out=losses_d[:], in_=CN[0:1, 0:8])
                return
            # ---------------- distance matrices ----------------
            for nt in ([] if skip_dist else range(16)):
                pa = psA.tile([128, N], F32, tag="big")
                for j in range(4):
                    nc.tensor.matmul(pa[:, _sl(j, 512)], R1[0:13, _sl(nt)],
                                     R2[0:13, _sl(j, 512)], start=True, stop=True)
                nc.vector.tensor_reduce(out=SP[:, MINA + nt:MINA + nt + 1],
                                        in_=pa[:, :], axis=AXL.X, op=ALU.min)
                pb = psA.tile([128, N], F32, tag="big")
                for j in range(4):
                    nc.tensor.matmul(pb[:, _sl(j, 512)], R1[32:45, _sl(nt)],
                                     R2[32:45, _sl(j, 512)], start=True, stop=True)
                nc.vector.tensor_reduce(out=SP[:, MINB + nt:MINB + nt + 1],
                                        in_=pb[:, :], axis=AXL.X, op=ALU.min)
                pc2 = psA.tile([128, N], F32, tag="big")
                for j in range(4):
                    nc.tensor.matmul(pc2[:, _sl(j, 512)], R1[0:13, _sl(nt)],
                                     R3[0:13, _sl(j, 512)], start=True, stop=True)
                nc.vector.max(out=SP[:, M8C + 8 * nt:M8C + 8 * nt + 8],
                              in_=pc2[:, :])

            if dbg:
                nc.sync.dma_start(out=minA_d[:], in_=spc(MINA, 16))
                nc.sync.dma_start(out=minB_d[:], in_=spc(MINB, 16))
                nc.sync.dma_start(out=m8_d[:], in_=spc(M8C, 128))

            # ---------------- per-core loss partials ----------------
            if skip_dist:
                nc.vector.memset(spc(MINA, 16), 1.0)
                nc.vector.memset(spc(MINB, 16), 1.0)
                nc.vector.memset(spc(M8C, 128), -1.0)
            nc.vector.tensor_scalar_max(spc(MINA, 16), spc(MINA, 16), 0.0)
            nc.vector.tensor_scalar_max(spc(MINB, 16), spc(MINB, 16), 0.0)
            nc.scalar.activation(out=spc(DA, 16), in_=spc(MINA, 16),
                                 func=AF.Sqrt, bias=CN[:, 2:3])
            nc.scalar.activation(out=spc(DB, 16), in_=spc(MINB, 16),
                                 func=AF.Sqrt, bias=CN[:, 2:3])
            nc.vector.memset(spc(PART, 8), 0.0)
            nc.vector.tensor_reduce(out=spc(PART + 0), in_=spc(DA, 16),
                                    axis=AXL.X, op=ALU.add)
            nc.vector.tensor_reduce(out=spc(PART + 1), in_=spc(DB, 16),
                                    axis=AXL.X, op=ALU.add)
            nc.vector.tensor_reduce(out=spc(HMX + 0), in_=spc(DA, 16),
                                    axis=AXL.X, op=ALU.max)
            nc.vector.tensor_reduce(out=spc(HMX + 1), in_=spc(DB, 16),
                                    axis=AXL.X, op=ALU.max)
            nc.vector.tensor_reduce(out=spc(PART + 5), in_=spc(HMX, 2),
                                    axis=AXL.X, op=ALU.max)
            # repulsion hinge on knn slots 1..5 of each row
            m8v = spc(M8C, 128).rearrange("p (a b) -> p a b", b=8)
            nc.vector.tensor_scalar(
                out=spc(KNN, 80).rearrange("p (a b) -> p a b", b=5),
                in0=m8v[:, :, 1:6], scalar1=-1.0, scalar2=0.0,
                op0=ALU.mult, op1=ALU.max)
            nc.scalar.activation(out=spc(KND, 80), in_=spc(KNN, 80),
                                 func=AF.Sqrt, bias=CN[:, 3:4])
            nc.scalar.activation(out=spc(HIN, 80), in_=spc(KND, 80),
                                 func=AF.Relu, scale=-1.0, bias=CN[:, 4:5])
            nc.vector.tensor_reduce(out=spc(PART + 2), in_=spc(HIN, 80),
                                    axis=AXL.X, op=ALU.add)
            nc.vector.tensor_copy(out=spc(PART + 3, 1, 3), in_=spc(MSEP, 1, 3))
            nc.vector.tensor_copy(out=spc(PART + 4, 1, 3), in_=spc(L1A, 1, 3))
            if dbg:
                nc.sync.dma_start(out=part_d[:], in_=spc(PART, 8))

            if cut == "part":
                nc.sync.dma_start(out=losses_d[:], in_=CN[0:1, 0:8])
                return
            # fold partitions via DRAM bounce transpose
            pp = dram.tile([128, 8], F32)
            nc.sync.dma_start(out=pp[:], in_=spc(PART, 8))
            ppb = pp[:]
            nc.sync.dma_start(out=SP[0:8, QC:QC + 128],
                              in_=bass.AP(tensor=ppb.tensor, offset=ppb.offset,
                                          ap=[[1, 8], [8, 128]]))
            nc.vector.tensor_reduce(out=spc(SUM8, 1, 8), in_=SP[0:8, QC:QC + 128],
                                    axis=AXL.X, op=ALU.add)
            nc.vector.tensor_reduce(out=spc(MAX8, 1, 8), in_=SP[0:8, QC:QC + 128],
                                    axis=AXL.X, op=ALU.max)
            # row 5 (per-core hausdorff max) comes from the max-reduce:
            # merged = sums + mask * (maxs - sums), mask = 1 only on row 5
            hm = PCOLS[("mask", "h", 0)]
            nc.vector.tensor_sub(spc(MAX8, 1, 8), spc(MAX8, 1, 8),
                                 spc(SUM8, 1, 8))
            nc.vector.tensor_mul(spc(MAX8, 1, 8), spc(MAX8, 1, 8),
                                 PAR[:8, hm:hm + 1])
            nc.vector.tensor_add(spc(SUM8, 1, 8), spc(SUM8, 1, 8),
                                 spc(MAX8, 1, 8))

            if cut == "sum8":
                nc.sync.dma_start(out=losses_d[:], in_=CN[0:1, 0:8])
                return
            ar_in = dram.tile([8, 1], F32)
            ar_out = dram.tile([8, 1], F32)
            nc.sync.dma_start(out=ar_in[:], in_=spc(SUM8, 1, 8))
            if nocc:
                nc.sync.dma_start(out=ar_out[:], in_=ar_in[:])
            else:
                nc.gpsimd.collective_compute(
                    "AllReduce", ALU.add, replica_groups=[list(range(N_CORES))],
                    ins=[ar_in[:].opt()], outs=[ar_out[:].opt()])
            arb = ar_out[:]
            nc.sync.dma_start(out=SP[0:1, TOT8:TOT8 + 8],
                              in_=bass.AP(tensor=arb.tensor, offset=arb.offset,
                                          ap=[[8, 1], [1, 8]]))

            if cut == "tot8":
                nc.sync.dma_start(out=losses_d[:], in_=CN[0:1, 0:8])
                return

            def T8(i):
                return SP[0:1, TOT8 + i:TOT8 + i + 1]

            def FN(i):
                return SP[0:1, FIN + i:FIN + i + 1]

            nc.vector.memset(SP[0:1, FIN:FIN + 8], 0.0)
            if cut == "f1":
                nc.sync.dma_start(out=losses_d[:], in_=CN[0:1, 0:8])
                return
            nc.vector.tensor_add(FN(1), T8(0), T8(1))
            nc.scalar.activation(out=FN(1), in_=FN(1), func=AF.Copy,
                                 scale=1.0 / 16384.0)
            nc.scalar.activation(out=FN(2), in_=T8(2), func=AF.Copy,
                                 scale=1.0 / 81920.0)
            nc.scalar.activation(out=SP[0:1, MSEV:MSEV + 1], in_=T8(3),
                                 func=AF.Copy, scale=1.0 / 49152.0)
            nc.scalar.activation(out=FN(3), in_=SP[0:1, MSEV:MSEV + 1],
                                 func=AF.Sqrt, bias=CN[0:1, 5:6])
            if cut == "f2":
                nc.sync.dma_start(out=losses_d[:], in_=CN[0:1, 0:8])
                return
            nc.scalar.activation(out=FN(0), in_=FN(3), func=AF.Copy, scale=10.0)
            nc.scalar.activation(out=FN(4), in_=T8(5), func=AF.Copy,
                                 scale=1.0 / 8.0)
            nc.scalar.activation(out=FN(5), in_=T8(4), func=AF.Copy,
                                 scale=1.0 / 49152.0)
            nc.sync.dma_start(out=losses_d[:], in_=SP[0:1, FIN:FIN + 8])

      _emit()
    nc.compile()
    return nc


_PROG_CACHE = {}


def _get_prog(dbg=False):
    if dbg not in _PROG_CACHE:
        _PROG_CACHE[dbg] = build_program(dbg)
    return _PROG_CACHE[dbg]


def make_in_maps(pc, coord, params):
    pc = np.asarray(pc, dtype=np.float32)
    coord = np.asarray(coord, dtype=np.float32)
    pr = {k: np.asarray(v, dtype=np.float32) for k, v in params.items()}

    w_all = np.zeros((128, WTOT), dtype=np.float32)

    def wput(nm, wt, ci, co):
        kch = (ci + 127) // 128
        for kc in range(kch):
            r = min(128, ci - kc * 128)
            w_all[:r, WOFF[nm] + kc * co:WOFF[nm] + (kc + 1) * co] = \
                wt[kc * 128:kc * 128 + r, :]

    wput("p0", pr["pW0"].T, 3, 64)
    wput("p1", pr["pW1"].T, 64, 128)
    wput("p2", pr["pW2"].T, 128, 1024)
    wput("c0", pr["cW0"][:, :64].T, 64, 512)
    wput("c0g", pr["cW0"][:, 64:].T, 1024, 512)
    wput("c1", pr["cW1"].T, 512, 256)
    wput("c2", pr["cW2"].T, 256, 128)
    wput("c3", pr["cW3"].T, 128, 3)

    par = np.zeros((128, NPC), dtype=np.float32)
    src = {"b": {"p": "pb", "c": "cb"}, "g": {"p": "pg", "c": "cg"},
           "be": {"p": "pbeta", "c": "cbeta"}}
    for nm, _, co, _ in LAYERS:
        mts = (co + 127) // 128
        for pfx in ("b", "g", "be"):
            vec = pr[src[pfx][nm[0]] + nm[1]]
            for mt in range(mts):
                mp = min(128, co - mt * 128)
                par[:mp, PCOLS[(pfx, nm, mt)]] = vec[mt * 128:mt * 128 + mp]
    par[:3, PCOLS[("b", "c3", 0)]] = pr["cb3"]
    par[5, PCOLS[("mask", "h", 0)]] = 1.0

    in_maps = []
    for b in range(N_CORES):
        m = {"w_all": w_all, "params": par,
             "pcT": np.ascontiguousarray(pc[b].T),
             "coordT": np.ascontiguousarray(coord[b].T)}
        in_maps.append(m)
    return in_maps


def build_sharded(nc, n_cores):
    """Build (once) a reusable jitted shard_map executable for the program."""
    import jax
    from jax.sharding import Mesh, PartitionSpec
    try:
        from jax.experimental.shard_map import shard_map
    except ImportError:
        from jax import shard_map
    import concourse.bass2jax as b2j
    b2j.install_neuronx_cc_hook()
    partition_name = (nc.partition_id_tensor.name
                      if nc.partition_id_tensor else None)
    in_names, out_names, out_avals, zero_outs = [], [], [], []
    for alloc in nc.m.functions[0].allocations:
        if not isinstance(alloc, mybir.MemoryLocationSet):
            continue
        name = alloc.memorylocations[0].name
        if alloc.kind == "ExternalInput":
            if name != partition_name:
                in_names.append(name)
        elif alloc.kind == "ExternalOutput":
            out_names.append(name)
            shape = tuple(alloc.tensor_shape)
            dtype = mybir.dt.np(alloc.dtype)
            out_avals.append(jax.core.ShapedArray(shape, dtype))
            zero_outs.append(np.zeros(shape, dtype))
    n_params = len(in_names)
    n_outs = len(out_avals)
    in_names_full = list(in_names) + out_names
    if partition_name:
        in_names_full.append(partition_name)
    donate = tuple(range(n_params, n_params + n_outs))

    def _body(*args):
        operands = list(args)
        if partition_name:
            operands.append(b2j.partition_id_tensor())
        outs = b2j._bass_exec_p.bind(
            *operands, out_avals=tuple(out_avals),
            in_names=tuple(in_names_full), out_names=tuple(out_names),
            lowering_input_output_aliases=(), sim_require_finite=True,
            sim_require_nnan=True, nc=nc)
        return tuple(outs)

    devices = jax.devices()[:n_cores]
    mesh = Mesh(np.asarray(devices), ("core",))
    in_specs = (PartitionSpec("core"),) * (n_params + n_outs)
    out_specs = (PartitionSpec("core"),) * n_outs
    sharded = jax.jit(shard_map(_body, mesh=mesh, in_specs=in_specs,
                                out_specs=out_specs, check_rep=False),
                      donate_argnums=donate, keep_unused=True)
    return sharded, mesh, in_names, out_names, zero_outs


_EXEC_CACHE = {}


def get_executable(dbg=False):
    if dbg not in _EXEC_CACHE:
        nc = _get_prog(dbg=dbg)
        _EXEC_CACHE[dbg] = build_sharded(nc, N_CORES)
    return _EXEC_CACHE[dbg]


def run(pc, coord, params, dbg=False):
    """Run the kernel; returns {output_name: (n_cores, ...) array}."""
    sharded, mesh, in_names, out_names, zero_outs = get_executable(dbg)
    in_maps = make_in_maps(pc, coord, params)
    concat_in = [np.concatenate([np.asarray(in_maps[c][nm])
                                 for c in range(N_CORES)], axis=0)
                 for nm in in_names]
    concat_zeros = [np.zeros((N_CORES * z.shape[0], *z.shape[1:]), z.dtype)
                    for z in zero_outs]
    out_arrs = sharded(*concat_in, *concat_zeros)
    res = {}
    for i, name in enumerate(out_names):
        a = np.asarray(out_arrs[i])
        res[name] = a.reshape(N_CORES, a.shape[0] // N_CORES, *a.shape[1:])
    return res


def kernel(pc, coord, params):
    res = run(pc, coord, params, dbg=False)
    return res["losses"][0].reshape(-1)[:6].astype(np.float32)


# revision 29
# speedup vs baseline: 3.6658x; 2.1774x over previous
"""Trainium2 Bass kernel for nn_EstCoordNet (PointNet-style loss_fn).

Sharding: data-parallel over B=8 across 8 NeuronCores (one batch item per
core).  BN batch stats are combined with one small AllGather per BN layer;
the final scalar losses with one AllReduce.  Convs run as plain fp32
matmuls; the three 2048x2048 distance matrices run as error-compensated
float32r matmuls (hi/lo split -> ~fp32 accuracy at full PE rate).

Key structural tricks:
  * activations kept channel-major (C, 2048) so BN scale/bias are
    per-partition vectors and one fused ACT Relu applies BN+relu.
  * global max-pool computed on pre-BN conv output (BN+relu is monotone
    for gamma>0), so the (1024, 2048) x3 tensor is never materialized and
    the g-channels of the c0 conv collapse to a per-core rank-1 bias
    v = Wg @ g folded into the BN shift.
  * repulsion needs each point's 5 smallest pred-pred distances: the PE
    emits -d2s and DVE Max8 yields the 8 largest per row in one pass.
"""
import sys

sys.path.insert(0, "/opt/trn_rl_repo")

import numpy as np

import concourse.bass as bass
import concourse.bacc as bacc
import concourse.tile as tile
import concourse.mybir as mybir

F32 = mybir.dt.float32
F32R = mybir.dt.float32r
AF = mybir.ActivationFunctionType
ALU = mybir.AluOpType
AXL = mybir.AxisListType

N_CORES = 8
N = 2048
M_TOT = float(N_CORES * N)
BN_EPS = 1e-5
REP_H, REP_EPS = 0.01, 1e-12

# (name, Cin, Cout, mode)
LAYERS = [
    ("p0", 3, 64, "hold"),
    ("p1", 64, 128, "hold"),
    ("p2", 128, 1024, "maxpool"),
    ("c0", 64, 512, "copy"),
    ("c1", 512, 256, "copy"),
    ("c2", 256, 128, "hold"),
]


def _sl(i, sz=128):
    return slice(i * sz, (i + 1) * sz)


def param_layout():
    cols, c = {}, 0
    for nm, _, co, _ in LAYERS:
        mts = (co + 127) // 128
        for pfx in ("b", "g", "be"):
            for mt in range(mts):
                cols[(pfx, nm, mt)] = c
                c += 1
    cols[("b", "c3", 0)] = c
    c += 1
    cols[("mask", "h", 0)] = c  # 1.0 on partition row 5, else 0
    c += 1
    return cols, c


PCOLS, NPC = param_layout()


def weight_layout():
    offs, c = {}, 0
    for nm, ci, co, _ in LAYERS:
        kch = (ci + 127) // 128
        offs[nm] = c
        c += kch * co
    offs["c0g"] = c
    c += 8 * 512
    offs["c3"] = c
    c += 3
    return offs, c


WOFF, WTOT = weight_layout()

# layer-stats tile column map (tile shape (128, 224))
SSQ, SYR, GMX, SV, TV, CV, T2C, VAR, MEAN, SX, ST2S, ST2Q, FLD, FLDQ = (
    0, 8, 16, 24, 32, 40, 48, 56, 64, 72, 80, 88, 96, 160)

# small_pack column map (tile shape (128, 640))
MINA, MINB, M8C, DA, DB, KNN, KND, HIN, PART, HMX = (
    0, 16, 32, 160, 176, 192, 272, 352, 432, 440)
QC, SUM8, MAX8, TOT8, FIN, MSEV, MSEP, L1A, L1B, SPC = (
    448, 576, 577, 578, 586, 594, 595, 596, 597, 598)


class _EarlyExit(Exception):
    pass


def build_program(dbg=False, nocc=False, skip_dist=False, layers_n=None, cut=None):
    nc = bacc.Bacc("TRN2", target_bir_lowering=False, debug=False,
                   num_devices=N_CORES)

    pcT_d = nc.dram_tensor("pcT", [3, N], F32R, kind="ExternalInput")
    coT_d = nc.dram_tensor("coordT", [3, N], F32, kind="ExternalInput")
    w_all_d = nc.dram_tensor("w_all", [128, WTOT], F32R, kind="ExternalInput")
    par_d = nc.dram_tensor("params", [128, NPC], F32, kind="ExternalInput")

    losses_d = nc.dram_tensor("losses", [1, 8], F32, kind="ExternalOutput")
    if dbg:
        pred_d = nc.dram_tensor("pred_dbg", [3, N], F32, kind="ExternalOutput")
        minA_d = nc.dram_tensor("minA_dbg", [128, 16], F32, kind="ExternalOutput")
        minB_d = nc.dram_tensor("minB_dbg", [128, 16], F32, kind="ExternalOutput")
        m8_d = nc.dram_tensor("m8_dbg", [128, 128], F32, kind="ExternalOutput")
        part_d = nc.dram_tensor("part_dbg", [128, 8], F32, kind="ExternalOutput")

    with tile.TileContext(nc) as tc:
      def _emit():
        import contextlib
        with contextlib.ExitStack() as ctx:
            io = ctx.enter_context(tc.tile_pool(name="io", bufs=1))
            yst = ctx.enter_context(tc.tile_pool(name="yst", bufs=6))
            aug = ctx.enter_context(tc.tile_pool(name="aug", bufs=1))
            lst = ctx.enter_context(tc.tile_pool(name="lst", bufs=2))
            scr = ctx.enter_context(tc.tile_pool(name="scr", bufs=3))
            scrr = ctx.enter_context(tc.tile_pool(name="scrr", bufs=2))
            psA = ctx.enter_context(tc.tile_pool(name="psA", bufs=2, space="PSUM"))
            dram = ctx.enter_context(tc.tile_pool(name="dram", bufs=1, space="DRAM"))

            # ---------------- load inputs ----------------
            pcT = io.tile([3, N], F32R)
            coT = io.tile([3, N], F32)
            nc.sync.dma_start(out=pcT[:], in_=pcT_d[:])
            nc.sync.dma_start(out=coT[:], in_=coT_d[:])
            WA = io.tile([128, WTOT], F32R)
            # split the 3.6MB weight load into ~256KB pieces so it spreads
            # across DMA queues and layers only wait for their own slice
            wcuts = []
            for nm, ci, co, _ in LAYERS:
                kch = (ci + 127) // 128
                for kc in range(kch):
                    wcuts.append(WOFF[nm] + kc * co)
            for kc in range(8):
                wcuts.append(WOFF["c0g"] + kc * 512)
            wcuts.append(WOFF["c3"])
            wcuts.append(WTOT)
            wcuts = sorted(set(wcuts))
            for a, b in zip(wcuts[:-1], wcuts[1:]):
                nc.sync.dma_start(out=WA[:, a:b], in_=w_all_d[:, a:b])
            PAR = io.tile([128, NPC], F32)
            nc.sync.dma_start(out=PAR[:], in_=par_d[:])

            def pv(pfx, nm, mt, mp):
                c = PCOLS[(pfx, nm, mt)]
                return PAR[:mp, c:c + 1]

            CN = io.tile([128, 8], F32)
            for col, val in [(0, 1.0), (1, BN_EPS), (2, 1e-24), (3, REP_EPS),
                             (4, REP_H), (5, 0.0), (6, -1.0)]:
                nc.vector.memset(CN[:, col:col + 1], val)

            SP = io.tile([128, 640], F32)  # small pack

            def spc(c0_, w=1, p=128):
                return SP[:p, c0_:c0_ + w]

            # f32r packs for the distance matmuls:
            #  R1: A_lhsT rows 0-12, B_lhsT rows 32-44
            #  R2: A_rhs  rows 0-12, B_rhs  rows 32-44
            #  R3: C_rhs  rows 0-12
            R1 = aug.tile([128, N], F32R)
            R2 = aug.tile([128, N], F32R)
            R3 = aug.tile([13, N], F32R)
            pred = aug.tile([3, N], F32)

            # ---------------- coord-side distance prep ----------------
            # A_rhs  = [c_hi;c_hi;c_lo;1;1;sc_hi;sc_lo]
            # B_lhsT = [-2c_hi;-2c_lo;-2c_hi;sc_hi;sc_lo;1;1]
            nc.vector.tensor_copy(out=R2[0:3, :], in_=coT[:])          # c_hi
            s_clo = scrr.tile([128, N], F32R, tag="bigr")
            nc.vector.tensor_sub(s_clo[0:3, :], coT[:], R2[0:3, :])    # c_lo
            nc.sync.dma_start(out=R2[3:6, :], in_=R2[0:3, :])
            nc.sync.dma_start(out=R2[6:9, :], in_=s_clo[0:3, :])
            nc.scalar.activation(out=R1[32:35, :], in_=R2[0:3, :],
                                 func=AF.Copy, scale=-2.0)             # -2c_hi
            s_m2clo = scrr.tile([128, N], F32R, tag="bigr")
            nc.scalar.activation(out=s_m2clo[0:3, :], in_=s_clo[0:3, :],
                                 func=AF.Copy, scale=-2.0)
            nc.sync.dma_start(out=R1[35:38, :], in_=s_m2clo[0:3, :])
            nc.sync.dma_start(out=R1[38:41, :], in_=R1[32:35, :])
            # ones row staged once, fanned out by DMA
            s_one = scr.tile([128, N], F32, tag="big")
            nc.vector.memset(s_one[0:1, :], 1.0)
            s_oner = scrr.tile([128, N], F32R, tag="bigr")
            nc.vector.tensor_copy(out=s_oner[0:1, :], in_=s_one[0:1, :])
            for dst, row in ((R2, 9), (R2, 10), (R1, 43), (R1, 44),
                             (R1, 11), (R1, 12), (R2, 41), (R2, 42)):
                nc.sync.dma_start(out=dst[row:row + 1, :], in_=s_oner[0:1, :])
            # sc = sum(coord^2) (fp32) via ones3 matmul
            s_sq = scr.tile([128, N], F32, tag="big")
            nc.scalar.activation(out=s_sq[0:3, :], in_=coT[:], func=AF.Square)
            psc = psA.tile([128, N], F32, tag="big")
            for j in range(4):
                nc.tensor.matmul(psc[0:1, _sl(j, 512)], CN[0:3, 0:1],
                                 s_sq[0:3, _sl(j, 512)], start=True, stop=True)
            s_sc = scr.tile([128, N], F32, tag="big")
            nc.scalar.copy(out=s_sc[0:1, :], in_=psc[0:1, :])
            s_schi = scrr.tile([128, N], F32R, tag="bigr")
            nc.vector.tensor_copy(out=s_schi[0:1, :], in_=s_sc[0:1, :])
            s_sclo = scrr.tile([128, N], F32R, tag="bigr")
            nc.vector.tensor_sub(s_sclo[0:1, :], s_sc[0:1, :], s_schi[0:1, :])
            nc.sync.dma_start(out=R2[11:12, :], in_=s_schi[0:1, :])
            nc.sync.dma_start(out=R2[12:13, :], in_=s_sclo[0:1, :])
            nc.sync.dma_start(out=R1[41:42, :], in_=s_schi[0:1, :])
            nc.sync.dma_start(out=R1[42:43, :], in_=s_sclo[0:1, :])

            # ---------------- forward ----------------
            nc.vector.tensor_reduce(out=spc(SPC, 1, 3), in_=pcT[:],
                                    axis=AXL.X, op=ALU.add)
            x_cur = [pcT]
            s_cur = [spc(SPC, 1, 3)]
            g_tiles = None
            x1_save = s1_save = None

            for nm, ci, co, mode in (LAYERS if layers_n is None else LAYERS[:layers_n]):
                kch = (ci + 127) // 128
                mts = (co + 127) // 128
                L = lst.tile([128, 224], F32, tag="L")

                def wsl(kc, mt, mp, _nm=nm, _co=co, _ci=ci):
                    c = WOFF[_nm] + kc * _co + mt * 128
                    return WA[:min(128, _ci - kc * 128), c:c + mp]

                # Sum-y via W @ s_x (transient psum tile, before conv rotation)
                if True:
                    ps1 = psA.tile([128, N], F32, tag="big")
                    for mt in range(mts):
                        mp = min(128, co - mt * 128)
                        for kc in range(kch):
                            nc.tensor.matmul(ps1[:mp, mt:mt + 1],
                                             wsl(kc, mt, mp).bitcast(F32),
                                             s_cur[kc], start=(kc == 0),
                                             stop=(kc == kch - 1))
                    nc.scalar.copy(out=L[:, SYR:SYR + mts], in_=ps1[:, 0:mts])
                if nm == "c0":
                    psv = psA.tile([128, N], F32, tag="big")
                    for mt in range(mts):
                        mp = min(128, co - mt * 128)
                        for kc in range(8):
                            c = WOFF["c0g"] + kc * 512 + mt * 128
                            nc.tensor.matmul(psv[:mp, mt:mt + 1],
                                             WA[:, c:c + mp].bitcast(F32),
                                             g_tiles[kc],
                                             start=(kc == 0), stop=(kc == 7))
                    nc.scalar.copy(out=L[:, CV:CV + mts], in_=psv[:, 0:mts])
                    for mt in range(mts):
                        mp = min(128, co - mt * 128)
                        nc.vector.tensor_add(L[:mp, CV + mt:CV + mt + 1],
                                             L[:mp, CV + mt:CV + mt + 1],
                                             pv("b", nm, mt, mp))

                y_ps, y_sb = [], []
                for mt in range(mts):
                    mp = min(128, co - mt * 128)
                    ps = psA.tile([128, N], F32, tag="big")
                    for kc in range(kch):
                        kr = min(128, ci - kc * 128)
                        for j in range(4):
                            xs = x_cur[kc][:kr, _sl(j, 512)]
                            if xs.dtype != F32:
                                xs = xs.bitcast(F32)
                            nc.tensor.matmul(
                                ps[:mp, _sl(j, 512)],
                                wsl(kc, mt, mp).bitcast(F32), xs,
                                start=(kc == 0), stop=(kc == kch - 1))
                    if mode == "copy":
                        # DVE: psum->sbuf rounded copy; ACT: square+accum from
                        # the copy (parallel engines). Sum-y comes from the
                        # W @ s_x matmul path like the other layers.
                        ysb = yst.tile([128, N], F32, tag="y")
                        nc.vector.tensor_copy(out=ysb[:mp, :], in_=ps[:mp, :])
                        sq_t = scr.tile([128, N], F32, tag="big")
                        nc.scalar.activation(out=sq_t[:mp, :], in_=ysb[:mp, :],
                                             func=AF.Square,
                                             accum_out=L[:mp, SSQ + mt:SSQ + mt + 1])
                        y_sb.append(ysb)
                    else:
                        sq_t = scr.tile([128, N], F32, tag="big")
                        nc.scalar.activation(out=sq_t[:mp, :], in_=ps[:mp, :],
                                             func=AF.Square,
                                             accum_out=L[:mp, SSQ + mt:SSQ + mt + 1])
                        y_ps.append(ps)
                    if mode == "maxpool":
                        nc.vector.tensor_reduce(out=L[:mp, GMX + mt:GMX + mt + 1],
                                                in_=ps[:mp, :], axis=AXL.X,
                                                op=ALU.max)
                        y_ps.pop()

                # ---- layer-wide (128, mts) stat math ----
                cvw = (L[:, CV:CV + mts] if nm == "c0"
                       else PAR[:, PCOLS[("b", nm, 0)]:PCOLS[("b", nm, 0)] + mts])
                syw = L[:, SYR:SYR + mts]
                ssqw = L[:, SSQ:SSQ + mts]
                sumc = L[:, ST2S:ST2S + mts]
                ssqc = L[:, ST2Q:ST2Q + mts]
                t2c = L[:, T2C:T2C + mts]
                # corrected stats: sy += N*c ; ssq += 2c*sy_raw + N*c^2
                nc.vector.scalar_tensor_tensor(
                    out=sumc, in0=cvw, scalar=float(N), in1=syw,
                    op0=ALU.mult, op1=ALU.add)
                nc.vector.scalar_tensor_tensor(
                    out=t2c, in0=cvw, scalar=2.0, in1=syw,
                    op0=ALU.mult, op1=ALU.mult)
                nc.vector.tensor_add(t2c, t2c, ssqw)
                sqv = L[:, VAR:VAR + mts]
                nc.vector.tensor_mul(sqv, cvw, cvw)
                nc.vector.scalar_tensor_tensor(
                    out=ssqc, in0=sqv, scalar=float(N), in1=t2c,
                    op0=ALU.mult, op1=ALU.add)

                # AllGather [sums(co); ssq(co)] across cores
                cc_in = dram.tile([2, co], F32, tag=f"ccin_{nm}")
                cc_out = dram.tile([2 * N_CORES, co], F32, tag=f"ccout_{nm}")
                cib = cc_in[:]
                for mt in range(mts):
                    mp = min(128, co - mt * 128)
                    nc.sync.dma_start(
                        out=bass.AP(tensor=cib.tensor,
                                    offset=cib.offset + mt * 128,
                                    ap=[[1, mp]]),
                        in_=L[:mp, ST2S + mt:ST2S + mt + 1])
                    nc.sync.dma_start(
                        out=bass.AP(tensor=cib.tensor,
                                    offset=cib.offset + co + mt * 128,
                                    ap=[[1, mp]]),
                        in_=L[:mp, ST2Q + mt:ST2Q + mt + 1])
                if nocc:
                    nc.sync.dma_start(out=cc_out[0:2, :], in_=cc_in[:])
                else:
                    nc.gpsimd.collective_compute(
                        "AllGather", ALU.bypass,
                        replica_groups=[list(range(N_CORES))],
                        ins=[cc_in[:].opt()], outs=[cc_out[:].opt()])
                base = cc_out[:]
                # fold across cores: per-mt DMAs, one wide reduce per block
                for blk, off in ((FLD, 0), (FLDQ, co)):
                    for mt in range(mts):
                        mp = min(128, co - mt * 128)
                        ap = bass.AP(tensor=base.tensor,
                                     offset=base.offset + off + mt * 128,
                                     ap=[[1, mp], [2 * co, N_CORES]])
                        nc.sync.dma_start(
                            out=L[:mp, blk + 8 * mt:blk + 8 * mt + 8], in_=ap)
                    fold = L[:, blk:blk + 8 * mts].rearrange(
                        "p (m b) -> p m b", b=N_CORES)
                    dst = sumc if off == 0 else ssqc
                    nc.vector.tensor_reduce(out=dst, in_=fold, axis=AXL.X,
                                            op=ALU.add)
                meanw = L[:, MEAN:MEAN + mts]
                nc.scalar.activation(out=meanw, in_=sumc, func=AF.Copy,
                                     scale=1.0 / M_TOT)
                nc.vector.scalar_tensor_tensor(
                    out=t2c, in0=meanw, scalar=-1.0, in1=meanw,
                    op0=ALU.mult, op1=ALU.mult)
                varw = L[:, VAR:VAR + mts]
                nc.vector.scalar_tensor_tensor(
                    out=varw, in0=ssqc, scalar=1.0 / M_TOT, in1=t2c,
                    op0=ALU.mult, op1=ALU.add)
                svw = L[:, SV:SV + mts]
                nc.scalar.activation(out=svw, in_=varw, func=AF.Sqrt,
                                     bias=CN[:, 1:2])
                nc.vector.reciprocal(out=svw, in_=svw)
                gcol = PCOLS[("g", nm, 0)]
                nc.vector.tensor_mul(svw, PAR[:, gcol:gcol + mts], svw)
                tvw = L[:, TV:TV + mts]
                nc.vector.tensor_sub(tvw, cvw, meanw)
                nc.vector.tensor_mul(tvw, tvw, svw)
                becol = PCOLS[("be", nm, 0)]
                nc.vector.tensor_add(tvw, tvw, PAR[:, becol:becol + mts])

                # apply
                if mode == "maxpool":
                    gw = L[:, GMX:GMX + mts]
                    nc.vector.tensor_mul(gw, gw, svw)
                    nc.vector.tensor_add(gw, gw, tvw)
                    nc.vector.tensor_scalar_max(gw, gw, 0.0)
                    g_tiles = [L[:, GMX + mt:GMX + mt + 1] for mt in range(mts)]
                    x_cur, s_cur = x1_save, s1_save
                else:
                    nx, ns = [], []
                    for mt in range(mts):
                        mp = min(128, co - mt * 128)
                        if mode == "copy":
                            xt = y_sb[mt]
                            src = xt[:mp, :]
                        else:
                            xt = yst.tile([128, N], F32, tag="y")
                            src = y_ps[mt][:mp, :]
                        sx = L[:mp, SX + mt:SX + mt + 1]
                        nc.scalar.activation(out=xt[:mp, :], in_=src,
                                             func=AF.Relu,
                                             scale=L[:mp, SV + mt:SV + mt + 1],
                                             bias=L[:mp, TV + mt:TV + mt + 1],
                                             accum_out=sx)
                        nx.append(xt)
                        ns.append(sx)
                    x_cur, s_cur = nx, ns
                    if nm == "p0":
                        x1_save, s1_save = nx, ns

            # ---------------- c3 -> pred ----------------
            if layers_n is not None:
                nc.sync.dma_start(out=losses_d[:], in_=CN[0:1, 0:8])
                return
            psp = psA.tile([128, N], F32, tag="big")
            c3c = WOFF["c3"]
            for j in range(4):
                nc.tensor.matmul(psp[:3, _sl(j, 512)],
                                 WA[:, c3c:c3c + 3].bitcast(F32),
                                 x_cur[0][:128, _sl(j, 512)], start=True, stop=True)
            nc.scalar.activation(out=pred[:], in_=psp[:3, :], func=AF.Identity,
                                 bias=pv("b", "c3", 0, 3))
            if dbg:
                nc.sync.dma_start(out=pred_d[:], in_=pred[:])
            if cut == "c3":
                nc.sync.dma_start(out=losses_d[:], in_=CN[0:1, 0:8])
                return

            # ---------------- pred-side distance prep ----------------
            # A_lhsT = [-2p_hi;-2p_lo;-2p_hi;sp_hi;sp_lo;1;1] @ R1 rows 0-12
            # B_rhs  = [p_hi;p_hi;p_lo;1;1;sp_hi;sp_lo]       @ R2 rows 32-44
            s_phi = scrr.tile([128, N], F32R, tag="bigr")
            nc.vector.tensor_copy(out=s_phi[0:3, :], in_=pred[:])      # p_hi
            nc.vector.tensor_copy(out=R2[32:35, :], in_=s_phi[0:3, :])
            nc.sync.dma_start(out=R2[35:38, :], in_=R2[32:35, :])
            s_plo = scrr.tile([128, N], F32R, tag="bigr")
            nc.vector.tensor_sub(s_plo[0:3, :], pred[:], s_phi[0:3, :])
            nc.sync.dma_start(out=R2[38:41, :], in_=s_plo[0:3, :])
            nc.scalar.activation(out=R1[0:3, :], in_=s_phi[0:3, :],
                                 func=AF.Copy, scale=-2.0)             # -2p_hi
            s_m2plo = scrr.tile([128, N], F32R, tag="bigr")
            nc.scalar.activation(out=s_m2plo[0:3, :], in_=s_plo[0:3, :],
                                 func=AF.Copy, scale=-2.0)
            nc.sync.dma_start(out=R1[3:6, :], in_=s_m2plo[0:3, :])
            nc.sync.dma_start(out=R1[6:9, :], in_=R1[0:3, :])
            # sp
            s_sqp = scr.tile([128, N], F32, tag="big")
            nc.scalar.activation(out=s_sqp[0:3, :], in_=pred[:], func=AF.Square)
            psq = psA.tile([128, N], F32, tag="big")
            for j in range(4):
                nc.tensor.matmul(psq[0:1, _sl(j, 512)], CN[0:3, 0:1],
                                 s_sqp[0:3, _sl(j, 512)], start=True, stop=True)
            s_sp = scr.tile([128, N], F32, tag="big")
            nc.scalar.copy(out=s_sp[0:1, :], in_=psq[0:1, :])
            s_sphi = scrr.tile([128, N], F32R, tag="bigr")
            nc.vector.tensor_copy(out=s_sphi[0:1, :], in_=s_sp[0:1, :])
            s_splo = scrr.tile([128, N], F32R, tag="bigr")
            nc.vector.tensor_sub(s_splo[0:1, :], s_sp[0:1, :], s_sphi[0:1, :])
            nc.sync.dma_start(out=R1[9:10, :], in_=s_sphi[0:1, :])
            nc.sync.dma_start(out=R1[10:11, :], in_=s_splo[0:1, :])
            nc.sync.dma_start(out=R2[43:44, :], in_=s_sphi[0:1, :])
            nc.sync.dma_start(out=R2[44:45, :], in_=s_splo[0:1, :])
            # C_rhs = -B_rhs
            nc.vector.tensor_scalar_mul(R3[0:13, :], R2[32:45, :], -1.0)

            if cut == "prep":
                nc.sync.dma_start(out=losses_d[:], in_=CN[0:1, 0:8])
                return
            # mse / smooth-l1 partials
            s_df = scr.tile([128, N], F32, tag="big")
            nc.vector.tensor_sub(s_df[0:3, :], pred[:], coT[:])
            s_t = scr.tile([128, N], F32, tag="big")
            nc.vector.scalar_tensor_tensor(
                out=s_t[0:3, :], in0=s_df[0:3, :], scalar=1.0, in1=s_df[0:3, :],
                op0=ALU.mult, op1=ALU.mult, accum_out=spc(MSEP, 1, 3))
            s_ab = scr.tile([128, N], F32, tag="big")
            nc.scalar.activation(out=s_ab[0:3, :], in_=s_df[0:3, :], func=AF.Abs)
            s_mn = scr.tile([128, N], F32, tag="big")
            nc.vector.tensor_scalar_min(s_mn[0:3, :], s_ab[0:3, :], 1.0)
            s_u = scr.tile([128, N], F32, tag="big")
            nc.vector.scalar_tensor_tensor(
                out=s_u[0:3, :], in0=s_mn[0:3, :], scalar=0.5, in1=s_mn[0:3, :],
                op0=ALU.mult, op1=ALU.mult, accum_out=spc(L1A, 1, 3))
            s_v2 = scr.tile([128, N], F32, tag="big")
            nc.scalar.activation(out=s_v2[0:3, :], in_=s_ab[0:3, :],
                                 func=AF.Relu, bias=CN[0:3, 6:7],
                                 accum_out=spc(L1B, 1, 3))
            nc.vector.tensor_add(spc(L1A, 1, 3), spc(L1A, 1, 3), spc(L1B, 1, 3))

            if cut == "mse":
                nc.sync.dma_start(out=losses_d[:], in_=CN[0:1, 0:8])
                return
            # ---------------- distance matrices ----------------
            for nt in ([] if skip_dist else range(16)):
                pa = psA.tile([128, N], F32, tag="big")
                for j in range(4):
                    nc.tensor.matmul(pa[:, _sl(j, 512)], R1[0:13, _sl(nt)],
                                     R2[0:13, _sl(j, 512)], start=True, stop=True)
                nc.vector.tensor_reduce(out=SP[:, MINA + nt:MINA + nt + 1],
                                        in_=pa[:, :], axis=AXL.X, op=ALU.min)
                pb = psA.tile([128, N], F32, tag="big")
                for j in range(4):
                    nc.tensor.matmul(pb[:, _sl(j, 512)], R1[32:45, _sl(nt)],
                                     R2[32:45, _sl(j, 512)], start=True, stop=True)
                nc.vector.tensor_reduce(out=SP[:, MINB + nt:MINB + nt + 1],
                                        in_=pb[:, :], axis=AXL.X, op=ALU.min)
                pc2 = psA.tile([128, N], F32, tag="big")
                for j in range(4):
                    nc.tensor.matmul(pc2[:, _sl(j, 512)], R1[0:13, _sl(nt)],
                                     R3[0:13, _sl(j, 512)], start=True, stop=True)
                nc.vector.max(out=SP[:, M8C + 8 * nt:M8C + 8 * nt + 8],
                              in_=pc2[:, :])

            if dbg:
                nc.sync.dma_start(out=minA_d[:], in_=spc(MINA, 16))
                nc.sync.dma_start(out=minB_d[:], in_=spc(MINB, 16))
                nc.sync.dma_start(out=m8_d[:], in_=spc(M8C, 128))

            # ---------------- per-core loss partials ----------------
            if skip_dist:
                nc.vector.memset(spc(MINA, 16), 1.0)
                nc.vector.memset(spc(MINB, 16), 1.0)
                nc.vector.memset(spc(M8C, 128), -1.0)
            nc.vector.tensor_scalar_max(spc(MINA, 16), spc(MINA, 16), 0.0)
            nc.vector.tensor_scalar_max(spc(MINB, 16), spc(MINB, 16), 0.0)
            nc.scalar.activation(out=spc(DA, 16), in_=spc(MINA, 16),
                                 func=AF.Sqrt, bias=CN[:, 2:3])
            nc.scalar.activation(out=spc(DB, 16), in_=spc(MINB, 16),
                                 func=AF.Sqrt, bias=CN[:, 2:3])
            nc.vector.memset(spc(PART, 8), 0.0)
            nc.vector.tensor_reduce(out=spc(PART + 0), in_=spc(DA, 16),
                                    axis=AXL.X, op=ALU.add)
            nc.vector.tensor_reduce(out=spc(PART + 1), in_=spc(DB, 16),
                                    axis=AXL.X, op=ALU.add)
            nc.vector.tensor_reduce(out=spc(HMX + 0), in_=spc(DA, 16),
                                    axis=AXL.X, op=ALU.max)
            nc.vector.tensor_reduce(out=spc(HMX + 1), in_=spc(DB, 16),
                                    axis=AXL.X, op=ALU.max)
            nc.vector.tensor_reduce(out=spc(PART + 5), in_=spc(HMX, 2),
                                    axis=AXL.X, op=ALU.max)
            # repulsion hinge on knn slots 1..5 of each row
            m8v = spc(M8C, 128).rearrange("p (a b) -> p a b", b=8)
            nc.vector.tensor_scalar(
                out=spc(KNN, 80).rearrange("p (a b) -> p a b", b=5),
                in0=m8v[:, :, 1:6], scalar1=-1.0, scalar2=0.0,
                op0=ALU.mult, op1=ALU.max)
            nc.scalar.activation(out=spc(KND, 80), in_=spc(KNN, 80),
                                 func=AF.Sqrt, bias=CN[:, 3:4])
            nc.scalar.activation(out=spc(HIN, 80), in_=spc(KND, 80),
                                 func=AF.Relu, scale=-1.0, bias=CN[:, 4:5])
            nc.vector.tensor_reduce(out=spc(PART + 2), in_=spc(HIN, 80),
                                    axis=AXL.X, op=ALU.add)
            nc.vector.tensor_copy(out=spc(PART + 3, 1, 3), in_=spc(MSEP, 1, 3))
            nc.vector.tensor_copy(out=spc(PART + 4, 1, 3), in_=spc(L1A, 1, 3))
            if dbg:
                nc.sync.dma_start(out=part_d[:], in_=spc(PART, 8))

            if cut == "part":
                nc.sync.dma_start(out=losses_d[:], in_=CN[0:1, 0:8])
                return
            # fold partitions via DRAM bounce transpose
            pp = dram.tile([128, 8], F32)
            nc.sync.dma_start(out=pp[:], in_=spc(PART, 8))
            ppb = pp[:]
            nc.sync.dma_start(out=SP[0:8, QC:QC + 128],
                              in_=bass.AP(tensor=ppb.tensor, offset=ppb.offset,
                                          ap=[[1, 8], [8, 128]]))
            nc.vector.tensor_reduce(out=spc(SUM8, 1, 8), in_=SP[0:8, QC:QC + 128],
                                    axis=AXL.X, op=ALU.add)
            nc.vector.tensor_reduce(out=spc(MAX8, 1, 8), in_=SP[0:8, QC:QC + 128],
                                    axis=AXL.X, op=ALU.max)
            # row 5 (per-core hausdorff max) comes from the max-reduce:
            # merged = sums + mask * (maxs - sums), mask = 1 only on row 5
            hm = PCOLS[("mask", "h", 0)]
            nc.vector.tensor_sub(spc(MAX8, 1, 8), spc(MAX8, 1, 8),
                                 spc(SUM8, 1, 8))
            nc.vector.tensor_mul(spc(MAX8, 1, 8), spc(MAX8, 1, 8),
                                 PAR[:8, hm:hm + 1])
            nc.vector.tensor_add(spc(SUM8, 1, 8), spc(SUM8, 1, 8),
                                 spc(MAX8, 1, 8))

            if cut == "sum8":
                nc.sync.dma_start(out=losses_d[:], in_=CN[0:1, 0:8])
                return
            ar_in = dram.tile([8, 1], F32)
            ar_out = dram.tile([8, 1], F32)
            nc.sync.dma_start(out=ar_in[:], in_=spc(SUM8, 1, 8))
            if nocc:
                nc.sync.dma_start(out=ar_out[:], in_=ar_in[:])
            else:
                nc.gpsimd.collective_compute(
                    "AllReduce", ALU.add, replica_groups=[list(range(N_CORES))],
                    ins=[ar_in[:].opt()], outs=[ar_out[:].opt()])
            arb = ar_out[:]
            nc.sync.dma_start(out=SP[0:1, TOT8:TOT8 + 8],
                              in_=bass.AP(tensor=arb.tensor, offset=arb.offset,
                                          ap=[[8, 1], [1, 8]]))

            if cut == "tot8":
                nc.sync.dma_start(out=losses_d[:], in_=CN[0:1, 0:8])
                return

            def T8(i):
                return SP[0:1, TOT8 + i:TOT8 + i + 1]

            def FN(i):
                return SP[0:1, FIN + i:FIN + i + 1]

            nc.vector.memset(SP[0:1, FIN:FIN + 8], 0.0)
            if cut == "f1":
                nc.sync.dma_start(out=losses_d[:], in_=CN[0:1, 0:8])
                return
            nc.vector.tensor_add(FN(1), T8(0), T8(1))
            nc.scalar.activation(out=FN(1), in_=FN(1), func=AF.Copy,
                                 scale=1.0 / 16384.0)
            nc.scalar.activation(out=FN(2), in_=T8(2), func=AF.Copy,
                                 scale=1.0 / 81920.0)
            nc.scalar.activation(out=SP[0:1, MSEV:MSEV + 1], in_=T8(3),
                                 func=AF.Copy, scale=1.0 / 49152.0)
            nc.scalar.activation(out=FN(3), in_=SP[0:1, MSEV:MSEV + 1],
                                 func=AF.Sqrt, bias=CN[0:1, 5:6])
            if cut == "f2":
                nc.sync.dma_start(out=losses_d[:], in_=CN[0:1, 0:8])
                return
            nc.scalar.activation(out=FN(0), in_=FN(3), func=AF.Copy, scale=10.0)
            nc.scalar.activation(out=FN(4), in_=T8(5), func=AF.Copy,
                                 scale=1.0 / 8.0)
            nc.scalar.activation(out=FN(5), in_=T8(4), func=AF.Copy,
                                 scale=1.0 / 49152.0)
            nc.sync.dma_start(out=losses_d[:], in_=SP[0:1, FIN:FIN + 8])

      _emit()
    nc.compile()
    return nc


_PROG_CACHE = {}


def _get_prog(dbg=False):
    if dbg not in _PROG_CACHE:
        _PROG_CACHE[dbg] = build_program(dbg)
    return _PROG_CACHE[dbg]


def make_in_maps(pc, coord, params):
    pc = np.asarray(pc, dtype=np.float32)
    coord = np.asarray(coord, dtype=np.float32)
    pr = {k: np.asarray(v, dtype=np.float32) for k, v in params.items()}

    w_all = np.zeros((128, WTOT), dtype=np.float32)

    def wput(nm, wt, ci, co):
        kch = (ci + 127) // 128
        for kc in range(kch):
            r = min(128, ci - kc * 128)
            w_all[:r, WOFF[nm] + kc * co:WOFF[nm] + (kc + 1) * co] = \
                wt[kc * 128:kc * 128 + r, :]

    wput("p0", pr["pW0"].T, 3, 64)
    wput("p1", pr["pW1"].T, 64, 128)
    wput("p2", pr["pW2"].T, 128, 1024)
    wput("c0", pr["cW0"][:, :64].T, 64, 512)
    wput("c0g", pr["cW0"][:, 64:].T, 1024, 512)
    wput("c1", pr["cW1"].T, 512, 256)
    wput("c2", pr["cW2"].T, 256, 128)
    wput("c3", pr["cW3"].T, 128, 3)

    par = np.zeros((128, NPC), dtype=np.float32)
    src = {"b": {"p": "pb", "c": "cb"}, "g": {"p": "pg", "c": "cg"},
           "be": {"p": "pbeta", "c": "cbeta"}}
    for nm, _, co, _ in LAYERS:
        mts = (co + 127) // 128
        for pfx in ("b", "g", "be"):
            vec = pr[src[pfx][nm[0]] + nm[1]]
            for mt in range(mts):
                mp = min(128, co - mt * 128)
                par[:mp, PCOLS[(pfx, nm, mt)]] = vec[mt * 128:mt * 128 + mp]
    par[:3, PCOLS[("b", "c3", 0)]] = pr["cb3"]
    par[5, PCOLS[("mask", "h", 0)]] = 1.0

    in_maps = []
    for b in range(N_CORES):
        m = {"w_all": w_all, "params": par,
             "pcT": np.ascontiguousarray(pc[b].T),
             "coordT": np.ascontiguousarray(coord[b].T)}
        in_maps.append(m)
    return in_maps


def build_sharded(nc, n_cores):
    """Build (once) a reusable jitted shard_map executable for the program."""
    import jax
    from jax.sharding import Mesh, PartitionSpec
    try:
        from jax.experimental.shard_map import shard_map
    except ImportError:
        from jax import shard_map
    import concourse.bass2jax as b2j
    b2j.install_neuronx_cc_hook()
    partition_name = (nc.partition_id_tensor.name
                      if nc.partition_id_tensor else None)
    in_names, out_names, out_avals, zero_outs = [], [], [], []
    for alloc in nc.m.functions[0].allocations:
        if not isinstance(alloc, mybir.MemoryLocationSet):
            continue
        name = alloc.memorylocations[0].name
        if alloc.kind == "ExternalInput":
            if name != partition_name:
                in_names.append(name)
        elif alloc.kind == "ExternalOutput":
            out_names.append(name)
            shape = tuple(alloc.tensor_shape)
            dtype = mybir.dt.np(alloc.dtype)
            out_avals.append(jax.core.ShapedArray(shape, dtype))
            zero_outs.append(np.zeros(shape, dtype))
    n_params = len(in_names)
    n_outs = len(out_avals)
    in_names_full = list(in_names) + out_names
    if partition_name:
        in_names_full.append(partition_name)
    donate = tuple(range(n_params, n_params + n_outs))

    def _body(*args):
        operands = list(args)
        if partition_name:
            operands.append(b2j.partition_id_tensor())
        outs = b2j._bass_exec_p.bind(
            *operands, out_avals=tuple(out_avals),
            in_names=tuple(in_names_full), out_names=tuple(out_names),
            lowering_input_output_aliases=(), sim_require_finite=True,
            sim_require_nnan=True, nc=nc)
        return tuple(outs)

    devices = jax.devices()[:n_cores]
    mesh = Mesh(np.asarray(devices), ("core",))
    in_specs = (PartitionSpec("core"),) * (n_params + n_outs)
    out_specs = (PartitionSpec("core"),) * n_outs
    sharded = jax.jit(shard_map(_body, mesh=mesh, in_specs=in_specs,
                                out_specs=out_specs, check_rep=False),
                      donate_argnums=donate, keep_unused=True)
    return sharded, mesh, in_names, out_names, zero_outs


_EXEC_CACHE = {}


def get_executable(dbg=False):
    if dbg not in _EXEC_CACHE:
        nc = _get_prog(dbg=dbg)
        _EXEC_CACHE[dbg] = build_sharded(nc, N_CORES)
    return _EXEC_CACHE[dbg]


def run(pc, coord, params, dbg=False):
    """Run the kernel; returns {output_name: (n_cores, ...) array}."""
    sharded, mesh, in_names, out_names, zero_outs = get_executable(dbg)
    in_maps = make_in_maps(pc, coord, params)
    concat_in = [np.concatenate([np.asarray(in_maps[c][nm])
                                 for c in range(N_CORES)], axis=0)
                 for nm in in_names]
    concat_zeros = [np.zeros((N_CORES * z.shape[0], *z.shape[1:]), z.dtype)
                    for z in zero_outs]
    out_arrs = sharded(*concat_in, *concat_zeros)
    res = {}
    for i, name in enumerate(out_names):
        a = np.asarray(out_arrs[i])
        res[name] = a.reshape(N_CORES, a.shape[0] // N_CORES, *a.shape[1:])
    return res


def kernel(pc, coord, params):
    res = run(pc, coord, params, dbg=False)
    return res["losses"][0].reshape(-1)[:6].astype(np.float32)
